# revision 1
# baseline (speedup 1.0000x reference)
"""Entmax attention Trainium2 kernel (8-core SPMD, head-parallel).

Math (matches the reference _entmax_naive exactly):
  q,k,v projections (fp32)  ->  scores = (q*scale) @ k^T  (fp32, causal)
  per row: k_support = #{j : s_j > tau*} where tau* solves sum relu(s - tau*) = 1
           (Newton iteration: 9 iters on bf16 scores + 2 fp32 polish iters,
            validated exact vs the sort-based reference on all 32768 rows)
  tau_star = (row_sum - 1)/k_support ; p = relu(s - tau_star) ; attn = p/sum(p)
  out = attn @ v ; final = out @ Wo^T  (per-core partial, summed on host)

Engine plan per Newton iteration (the hot loop): every unit needs
F = sum relu(s-t) and cnt = #{s>t}. DVE reduce-ops run at 1x mode, so the
work is split half/half with ACT: half the units run F on ACT
(Relu+accumulate) and cnt on DVE (is_gt+accumulate); the other half run
F on DVE (scalar_tensor_tensor) and cnt on ACT via the Sign trick:
sum sign(s-t) = cnt - (n-cnt)  =>  cnt = 0.5*A + n/2.

attn probs are written as fp16 and transposed by the DMA xbar engines
(no PE/DVE cost); attn @ v runs in fp16 (out rel err ~3e-4 vs fp32 ref).

Sharding: 16 heads / 8 cores = 2 heads per core. Each core computes its
heads' attention output and the partial Wo product [2048,1024]; the host
sums the 8 partials.
"""
import numpy as np
from contextlib import ExitStack

import concourse.bass as bass
import concourse.tile as tile
import concourse.mybir as mybir
from concourse import bacc
from concourse.bass_utils import run_bass_kernel_spmd

L = 2048
D = 1024
H = 16
HD = 64
N_CORES = 8
HPC = 2  # heads per core
SCALE = float(HD) ** -0.5

FP32 = mybir.dt.float32
FP16 = mybir.dt.float16
BF16 = mybir.dt.bfloat16
Alu = mybir.AluOpType
Act = mybir.ActivationFunctionType

N_BF16_ITERS = 9
N_F32_ITERS = 2
NEG_BIG = -1.0e30
MAX_INIT = -3.0e38

# pairs (a, b) with (a+1)+(b+1) = 17 -> constant pair width 2176
RB_PAIRS = [(0, 15), (4, 11), (1, 14), (5, 10), (2, 13), (6, 9), (3, 12), (7, 8)]
PAIR_W = 17 * 128  # 2176


def _units_of_group(g):
    """8 units: (rb, head, slot, col_off). Order: h0 units first (cols 0-3
    of the group's stats slice), then h1 units (cols 4-7), so the engine
    split (F-on-ACT for first half, Sign-cnt for second half) uses
    contiguous stats columns. slot = 2*pair_local + h indexes the S tile."""
    units = []
    p0, p1 = RB_PAIRS[2 * g], RB_PAIRS[2 * g + 1]
    for h in range(HPC):
        for pi_local, (ra, rb_) in enumerate((p0, p1)):
            slot = 2 * pi_local + h
            na = 128 * (ra + 1)
            units.append((ra, h, slot, 0))
            units.append((rb_, h, slot, na))
    return units


def build_program(n_groups=4, do_newton=True, do_avwo=True, debug_out=None):
    nc = bacc.Bacc("TRN2", target_bir_lowering=False, debug=False, num_devices=1)

    xT_d = nc.dram_tensor("xT", [D, L], FP32, kind="ExternalInput")
    wq_d = nc.dram_tensor("wqT", [D, 128], FP32, kind="ExternalInput")
    wk_d = nc.dram_tensor("wkT", [D, 128], FP32, kind="ExternalInput")
    wv_d = nc.dram_tensor("wvT", [D, 128], FP32, kind="ExternalInput")
    wo_d = nc.dram_tensor("woT", [128, D], FP32, kind="ExternalInput")
    mneg_d = nc.dram_tensor("mneg", [128, 128], FP32, kind="ExternalInput")
    m01_d = nc.dram_tensor("m01", [128, 128], FP32, kind="ExternalInput")
    ident_d = nc.dram_tensor("ident", [128, 128], FP32, kind="ExternalInput")
    out_d = nc.dram_tensor("out", [L, D], FP32, kind="ExternalOutput")

    with tile.TileContext(nc) as tc:
        with ExitStack() as ctx:
            # ---------- persistent pools ----------
            persist = ctx.enter_context(tc.tile_pool(name="persist", bufs=1))
            qT = persist.tile([128, L], FP32, tag="qT")        # [d(2 heads), i]
            kT = persist.tile([128, L], FP32, tag="kT")        # [d(2 heads), j]
            vt = persist.tile([128, 16, 64 * HPC], FP16, tag="vt")  # [j, jt, d]
            woT = persist.tile([128, D], FP32, tag="woT")      # [d, o]
            mneg = persist.tile([128, 128], FP32, tag="mneg")
            m01 = persist.tile([128, 128], FP32, tag="m01")
            ident = persist.tile([128, 128], FP32, tag="ident")
            ident_h = persist.tile([128, 128], FP16, tag="identh")
            zeros_bf = persist.tile([128, L], BF16, tag="zbf")
            trash_a = persist.tile([128, L], BF16, tag="tra")   # ACT sink
            trash_d = persist.tile([128, L], BF16, tag="trd")   # DVE F sink
            trash_c = persist.tile([128, L], BF16, tag="trc")   # DVE cnt sink

            NST = 32

            def stat(tag):
                return persist.tile([128, NST], FP32, tag=tag, name=tag)

            maxF, maxD = stat("maxF"), stat("maxD")
            sumF, sumD = stat("sumF"), stat("sumD")
            mx, sm = stat("mx"), stat("sm")
            Tt, nT = stat("T"), stat("nT")
            Ft, Ct = stat("F"), stat("C")
            rec, Fm, dlt = stat("rec"), stat("Fm"), stat("dlt")
            tau, ntau = stat("tau"), stat("ntau")
            sump, rz = stat("sump"), stat("rz")
            nh = stat("nh")   # per-column n/2 for the Sign-count fixup

            nc.sync.dma_start(mneg[:], mneg_d.ap())
            nc.sync.dma_start(m01[:], m01_d.ap())
            nc.sync.dma_start(ident[:], ident_d.ap())
            nc.scalar.copy(ident_h[:], ident[:])
            nc.sync.dma_start(woT[:], wo_d.ap())
            nc.vector.memset(zeros_bf[:], 0.0)
            nc.vector.memset(maxF[:], MAX_INIT)
            nc.vector.memset(sumF[:], 0.0)
            nc.vector.memset(maxD[:], MAX_INIT)
            nc.vector.memset(sumD[:], 0.0)
            for g in range(n_groups):
                for ui, (rb, h, slot, off) in enumerate(_units_of_group(g)):
                    col = 8 * g + ui
                    nc.vector.memset(nh[:, col:col + 1], 64.0 * (rb + 1))

            # ---------- phase 1: projections ----------
            with ExitStack() as p1:
                ph1 = p1.enter_context(tc.tile_pool(name="ph1", bufs=1))
                ph1p = p1.enter_context(
                    tc.tile_pool(name="ph1p", bufs=2, space="PSUM"))
                xt = ph1.tile([128, 8, L], FP32, tag="xt")
                wqs = ph1.tile([128, 8, 128], FP32, tag="wqs")
                wks = ph1.tile([128, 8, 128], FP32, tag="wks")
                wvs = ph1.tile([128, 8, 128], FP32, tag="wvs")

                xview = xT_d.ap().rearrange("(c p) n -> p c n", p=128)
                for c in range(8):
                    nc.sync.dma_start(xt[:, c, :], xview[:, c, :])
                nc.sync.dma_start(wqs[:], wq_d.ap().rearrange("(c p) m -> p c m", p=128))
                nc.sync.dma_start(wks[:], wk_d.ap().rearrange("(c p) m -> p c m", p=128))
                nc.sync.dma_start(wvs[:], wv_d.ap().rearrange("(c p) m -> p c m", p=128))

                for dst, wsb in ((qT, wqs), (kT, wks)):
                    for ic in range(4):
                        ps = ph1p.tile([128, 512], FP32, tag="pp")
                        for e in range(8):
                            nc.tensor.matmul(
                                ps[:], wsb[:, e, :], xt[:, e, 512 * ic:512 * (ic + 1)],
                                start=(e == 0), stop=(e == 7))
                        if ic % 2 == 0:
                            nc.scalar.copy(dst[:, 512 * ic:512 * (ic + 1)], ps[:])
                        else:
                            nc.vector.tensor_copy(dst[:, 512 * ic:512 * (ic + 1)], ps[:])
                for jt in range(16):
                    ps = ph1p.tile([128, 512], FP32, tag="pp")
                    for e in range(8):
                        nc.tensor.matmul(
                            ps[:, :128], xt[:, e, 128 * jt:128 * (jt + 1)], wvs[:, e, :],
                            start=(e == 0), stop=(e == 7))
                    if jt % 2 == 0:
                        nc.scalar.copy(vt[:, jt, :], ps[:, :128])
                    else:
                        nc.vector.tensor_copy(vt[:, jt, :], ps[:, :128])

            if debug_out == "qkv":
                flat = out_d.ap().rearrange("a b -> (a b)")
                nc.sync.dma_start(flat[0:262144], qT[:])
                nc.sync.dma_start(flat[262144:524288], kT[:])

            # ---------- phase 2 pools ----------
            s_pool = ctx.enter_context(tc.tile_pool(name="spair", bufs=2))
            sb_pool = ctx.enter_context(tc.tile_pool(name="sbpair", bufs=2))
            p_pool = ctx.enter_context(tc.tile_pool(name="ppair", bufs=1))
            ps_sc = ctx.enter_context(tc.tile_pool(name="ps_sc", bufs=1, space="PSUM"))
            ps_av = ctx.enter_context(tc.tile_pool(name="ps_av", bufs=2, space="PSUM"))
            ps_tr = ctx.enter_context(tc.tile_pool(name="ps_tr", bufs=2, space="PSUM"))
            ptb_pool = ctx.enter_context(tc.tile_pool(name="ptb", bufs=2))
            oc_pool = ctx.enter_context(tc.tile_pool(name="oc", bufs=2))
            wo_pool = ctx.enter_context(tc.tile_pool(name="woout", bufs=2))

            copy_flip = [0]

            def balanced_copy(dst, src):
                if copy_flip[0] % 2 == 0:
                    nc.scalar.copy(dst, src)
                else:
                    nc.vector.tensor_copy(dst, src)
                copy_flip[0] += 1

            for g in range(n_groups):
                units = _units_of_group(g)
                gsl = slice(8 * g, 8 * g + 8)
                hsl = slice(8 * g + 4, 8 * g + 8)   # sign-cnt columns
                Sg = [s_pool.tile([128, PAIR_W], FP32, tag=f"sp{s}", name=f"sp{s}_{g}")
                      for s in range(4)]
                Sbg = [sb_pool.tile([128, PAIR_W], BF16, tag=f"sb{s}", name=f"sb{s}_{g}")
                       for s in range(4)]

                # ---- A/B: scores -> S (fp32, masked), Sb (bf16), sums/maxes ----
                for ui, (rb, h, slot, off) in enumerate(units):
                    col = 8 * g + ui
                    n = 128 * (rb + 1)
                    full = n - 128
                    S, Sb = Sg[slot], Sbg[slot]
                    ps = ps_sc.tile([128, 2048], FP32, tag="sc", name=f"sc{g}_{ui}")
                    for c0 in range(0, n, 512):
                        w = min(512, n - c0)
                        nc.tensor.matmul(
                            ps[:, c0:c0 + w],
                            qT[64 * h:64 * h + 64, 128 * rb:128 * rb + 128],
                            kT[64 * h:64 * h + 64, c0:c0 + w],
                            start=True, stop=True)
                    if full > 0:
                        # psum -> Sb (bf16) + row-sum (ACT)
                        nc.scalar.activation(
                            Sb[:, off:off + full], ps[:, :full], Act.Identity,
                            bias=0.0, accum_out=sumF[:, col:col + 1])
                        # psum -> S (fp32) + row-max (DVE)
                        nc.vector.tensor_scalar(
                            out=S[:, off:off + full], in0=ps[:, :full],
                            scalar1=0.0, scalar2=MAX_INIT,
                            op0=Alu.add, op1=Alu.max,
                            accum_out=maxF[:, col:col + 1])
                    # diag: mask to -1e30 into S, then row-max
                    nc.vector.tensor_tensor(
                        S[:, off + full:off + n], ps[:, full:n], mneg[:], Alu.add)
                    nc.vector.tensor_scalar(
                        out=trash_c[:, :128], in0=S[:, off + full:off + n],
                        scalar1=0.0, scalar2=MAX_INIT,
                        op0=Alu.add, op1=Alu.max,
                        accum_out=maxD[:, col:col + 1])
                    # diag row-sum of valid entries
                    nc.vector.scalar_tensor_tensor(
                        out=trash_d[:, :128],
                        in0=ps[:, full:n], scalar=1.0, in1=m01[:],
                        op0=Alu.mult, op1=Alu.mult,
                        accum_out=sumD[:, col:col + 1])
                    # masked diag -> Sb
                    nc.vector.tensor_copy(Sb[:, off + full:off + n],
                                          S[:, off + full:off + n])

                # ---- combine stats, init T/negT ----
                nc.vector.tensor_tensor(mx[:, gsl], maxF[:, gsl], maxD[:, gsl], Alu.max)
                nc.vector.tensor_tensor(sm[:, gsl], sumF[:, gsl], sumD[:, gsl], Alu.add)
                nc.vector.tensor_scalar_add(Tt[:, gsl], mx[:, gsl], -1.0)
                nc.vector.tensor_scalar(
                    out=nT[:, gsl], in0=mx[:, gsl], scalar1=-1.0, scalar2=1.0,
                    op0=Alu.mult, op1=Alu.add)

                if debug_out == "scores":
                    flat2 = out_d.ap().rearrange("a b -> (a b)")
                    for slot in range(4):
                        nc.sync.dma_start(
                            flat2[278528 * slot:278528 * (slot + 1)], Sg[slot][:])
                    continue
                if not do_newton:
                    continue

                # ---- Newton iterations ----
                def emit_passes(use_bf16, skip_F=False):
                    """First-half units: F on ACT (Relu), cnt on DVE (is_gt).
                    Second-half units: F on DVE (stt), cnt on ACT (Sign)."""
                    for ui, (rb, h, slot, off) in enumerate(units):
                        col = 8 * g + ui
                        n = 128 * (rb + 1)
                        Ssrc = Sbg[slot] if use_bf16 else Sg[slot]
                        if ui < 4:
                            if not skip_F:
                                nc.scalar.activation(
                                    trash_a[:, :n], Ssrc[:, off:off + n], Act.Relu,
                                    bias=nT[:, col:col + 1],
                                    accum_out=Ft[:, col:col + 1])
                            nc.vector.tensor_scalar(
                                out=trash_c[:, :n], in0=Ssrc[:, off:off + n],
                                scalar1=Tt[:, col:col + 1], scalar2=0.0,
                                op0=Alu.is_gt, op1=Alu.add,
                                accum_out=Ct[:, col:col + 1])
                        else:
                            if not skip_F:
                                nc.vector.scalar_tensor_tensor(
                                    out=trash_d[:, :n], in0=Ssrc[:, off:off + n],
                                    scalar=nT[:, col:col + 1], in1=zeros_bf[:, :n],
                                    op0=Alu.add, op1=Alu.max,
                                    accum_out=Ft[:, col:col + 1])
                            nc.scalar.activation(
                                trash_a[:, :n], Ssrc[:, off:off + n], Act.Sign,
                                bias=nT[:, col:col + 1],
                                accum_out=Ct[:, col:col + 1])
                    # sign-cnt fixup: cnt = 0.5*A + n/2  (columns 4-7)
                    nc.vector.scalar_tensor_tensor(
                        out=Ct[:, hsl], in0=Ct[:, hsl], scalar=0.5, in1=nh[:, hsl],
                        op0=Alu.mult, op1=Alu.add)

                def newton_update():
                    nc.vector.tensor_scalar_max(Ct[:, gsl], Ct[:, gsl], 1.0)
                    nc.vector.reciprocal(rec[:, gsl], Ct[:, gsl])
                    nc.vector.tensor_scalar_add(Fm[:, gsl], Ft[:, gsl], -1.0)
                    nc.vector.tensor_tensor(dlt[:, gsl], Fm[:, gsl], rec[:, gsl], Alu.mult)
                    nc.vector.tensor_tensor(Tt[:, gsl], Tt[:, gsl], dlt[:, gsl], Alu.add)
                    nc.vector.tensor_tensor(nT[:, gsl], nT[:, gsl], dlt[:, gsl], Alu.subtract)

                for _ in range(N_BF16_ITERS):
                    emit_passes(True)
                    newton_update()
                for _ in range(N_F32_ITERS):
                    emit_passes(False)
                    newton_update()

                # ---- final count -> k_support; tau_star ----
                emit_passes(False, skip_F=True)
                nc.vector.tensor_scalar_max(Ct[:, gsl], Ct[:, gsl], 1.0)
                nc.vector.reciprocal(rec[:, gsl], Ct[:, gsl])
                nc.vector.tensor_scalar_add(Fm[:, gsl], sm[:, gsl], -1.0)
                nc.vector.tensor_tensor(tau[:, gsl], Fm[:, gsl], rec[:, gsl], Alu.mult)
                nc.vector.tensor_scalar_mul(ntau[:, gsl], tau[:, gsl], -1.0)

                if not do_avwo:
                    continue

                # ---- p = relu(S - tau) (fp16) + row sum; AV (fp16); Wo ----
                Pg = [p_pool.tile([128, PAIR_W], FP16, tag=f"pp{s}", name=f"pp{s}_{g}")
                      for s in range(4)]
                outc_of_rb = {}
                for ui, (rb, h, slot, off) in enumerate(units):
                    col = 8 * g + ui
                    n = 128 * (rb + 1)
                    S, P = Sg[slot], Pg[slot]
                    nc.scalar.activation(
                        P[:, off:off + n], S[:, off:off + n], Act.Relu,
                        bias=ntau[:, col:col + 1],
                        accum_out=sump[:, col:col + 1])
                    # rz = 1/(sump + 1e-10)
                    nc.vector.tensor_scalar_add(
                        Fm[:, col:col + 1], sump[:, col:col + 1], 1.0e-10)
                    nc.vector.reciprocal(rz[:, col:col + 1], Fm[:, col:col + 1])

                    if rb not in outc_of_rb:
                        outc_of_rb[rb] = oc_pool.tile(
                            [128, 128], FP32, tag=f"oc{ui % 2}", name=f"oc{g}_{rb}")
                    outc = outc_of_rb[rb]

                    av = ps_av.tile([128, 512], FP32, tag="av", name=f"av{g}_{ui}")
                    nt = n // 128
                    for c0 in range(0, nt, 4):
                        cw = min(4, nt - c0)
                        pt_ps = ps_tr.tile([128, 512], FP16, tag="tr",
                                           name=f"ptp{g}_{ui}_{c0}")
                        for c in range(cw):
                            jt = c0 + c
                            nc.tensor.transpose(
                                pt_ps[:, 128 * c:128 * (c + 1)],
                                P[:, off + 128 * jt:off + 128 * (jt + 1)],
                                ident_h[:])
                        pt_sb = ptb_pool.tile([128, 512], FP16, tag="ptb",
                                              name=f"ptb{g}_{ui}_{c0}")
                        balanced_copy(pt_sb[:, :128 * cw], pt_ps[:, :128 * cw])
                        for c in range(cw):
                            jt = c0 + c
                            nc.tensor.matmul(
                                av[:, :64], pt_sb[:, 128 * c:128 * (c + 1)],
                                vt[:, jt, 64 * h:64 * h + 64],
                                start=(jt == 0), stop=(jt == nt - 1))
                    # normalize while copying out of psum
                    nc.scalar.activation(
                        outc[:, 64 * h:64 * h + 64], av[:, :64], Act.Copy,
                        bias=0.0, scale=rz[:, col:col + 1])

                    if h == 1:
                        # both heads done -> Wo partial for this rb
                        wo_out = wo_pool.tile([128, D], FP32, tag="wod",
                                              name=f"wod{g}_{rb}")
                        otb = ptb_pool.tile([128, 512], FP32, tag="otb",
                                            name=f"otb{g}_{rb}")
                        # transpose outc (fp32 [128,128]) via two 64-part DMAs
                        # -- not supported for 4B; use PE-free path: DVE 32x32
                        # block transpose is wrong; keep PE transpose here
                        # (only 16 per core).
                        wps_t = ps_av.tile([128, 512], FP32, tag="av",
                                           name=f"ot{g}_{rb}")
                        nc.tensor.transpose(wps_t[:, :128], outc[:], ident[:])
                        balanced_copy(otb[:, :128], wps_t[:, :128])
                        for oc2 in range(2):
                            wps = ps_av.tile([128, 512], FP32, tag="av",
                                             name=f"wo{g}_{rb}_{oc2}")
                            nc.tensor.matmul(
                                wps[:], otb[:, :128], woT[:, 512 * oc2:512 * (oc2 + 1)],
                                start=True, stop=True)
                            balanced_copy(wo_out[:, 512 * oc2:512 * (oc2 + 1)], wps[:])
                        nc.sync.dma_start(
                            out_d.ap()[128 * rb:128 * (rb + 1), :], wo_out[:])

    nc.compile()
    return nc


_CACHE = {}


def _get_nc():
    if "nc" not in _CACHE:
        _CACHE["nc"] = build_program()
    return _CACHE["nc"]


def _host_inputs(x, Wq, Wk, Wv, Wo):
    xT = np.ascontiguousarray(x[0].T).astype(np.float32)
    ii = np.arange(128)
    mneg = np.where(ii[None, :] > ii[:, None], np.float32(NEG_BIG),
                    np.float32(0.0)).astype(np.float32)
    m01 = (ii[None, :] <= ii[:, None]).astype(np.float32)
    in_maps = []
    for c in range(N_CORES):
        hsl = slice(128 * c, 128 * (c + 1))
        in_maps.append({
            "xT": xT,
            "wqT": np.ascontiguousarray((Wq[hsl] * np.float32(SCALE)).T).astype(np.float32),
            "wkT": np.ascontiguousarray(Wk[hsl].T).astype(np.float32),
            "wvT": np.ascontiguousarray(Wv[hsl].T).astype(np.float32),
            "woT": np.ascontiguousarray(Wo[:, hsl].T).astype(np.float32),
            "mneg": mneg,
            "m01": m01,
            "ident": np.eye(128, dtype=np.float32),
        })
    return in_maps


def kernel(x, Wq, Wk, Wv, Wo, _trace=False):
    nc = _get_nc()
    in_maps = _host_inputs(np.asarray(x), np.asarray(Wq), np.asarray(Wk),
                           np.asarray(Wv), np.asarray(Wo))
    res = run_bass_kernel_spmd(nc, in_maps, core_ids=list(range(N_CORES)),
                               trace=_trace)
    out = np.zeros((L, D), np.float32)
    for c in range(N_CORES):
        out += res.results[c]["out"]
    if _trace:
        _CACHE["last_results"] = res
    return out.reshape(1, L, D)



# revision 4
# speedup vs baseline: 1.7331x; 1.7331x over previous
"""Entmax attention Trainium2 kernel v4 (8-core SPMD, head-parallel).

vs v2: fp32r dropped (HW rounding broke k_support exactness) -- q/k/score
matmuls are plain fp32. P^T and outc^T via DMA xbar transposes (no PE
transpose passes, no psum->sbuf copies). v projection via fp16 vT route.
Newton: 8 overrelaxed fp32 iterations (omega 2.0,1.6,1.3,1.1,1,1,1,1) + final
count; cnt on DVE (tensor_scalar is_gt accum, fp32 2x_2p), F on ACT (6 units)
+ DVE stt (2 units); stats on DVE.

v4: top-8 (nc.vector.max) per row seeds T0 = max_k (cumsum_k - 1)/k -- the
running-tau peak of the sorted top-8. 73%% of rows are exact at T0; plain
Newton reaches 100%% exact k_support in 3 iterations (4 used for margin).
Replaces the 8 overrelaxed iterations and the phase-A row-max pass.
"""
import numpy as np
from contextlib import ExitStack

import concourse.bass as bass
import concourse.tile as tile
import concourse.mybir as mybir
from concourse import bacc
from concourse.bass_utils import run_bass_kernel_spmd

L = 2048
D = 1024
H = 16
HD = 64
N_CORES = 8
HPC = 2
SCALE = float(HD) ** -0.5

FP32 = mybir.dt.float32
FP16 = mybir.dt.float16
BF16 = mybir.dt.bfloat16
Alu = mybir.AluOpType
Act = mybir.ActivationFunctionType

N_ITERS = 4
NEG_BIG = -1.0e30
MAX_INIT = -3.0e38

RB_PAIRS = [(0, 15), (4, 11), (1, 14), (5, 10), (2, 13), (6, 9), (3, 12), (7, 8)]
PAIR_W = 17 * 128  # 2176


def _units_of_group(g):
    units = []
    p0, p1 = RB_PAIRS[2 * g], RB_PAIRS[2 * g + 1]
    for h in range(HPC):
        for pi_local, (ra, rb_) in enumerate((p0, p1)):
            slot = 2 * pi_local + h
            na = 128 * (ra + 1)
            units.append((ra, h, slot, 0))
            units.append((rb_, h, slot, na))
    return units


def build_program(n_groups=4):
    nc = bacc.Bacc("TRN2", target_bir_lowering=False, debug=False, num_devices=1)

    xT_d = nc.dram_tensor("xT", [D, L], FP32, kind="ExternalInput")
    wq_d = nc.dram_tensor("wqT", [D, 128], FP32, kind="ExternalInput")
    wk_d = nc.dram_tensor("wkT", [D, 128], FP32, kind="ExternalInput")
    wv_d = nc.dram_tensor("wvT", [D, 128], FP32, kind="ExternalInput")
    wo_d = nc.dram_tensor("woT", [128, D], FP32, kind="ExternalInput")
    mneg_d = nc.dram_tensor("mneg", [128, 128], FP32, kind="ExternalInput")
    m01_d = nc.dram_tensor("m01", [128, 128], FP32, kind="ExternalInput")
    ident_d = nc.dram_tensor("ident", [128, 128], FP32, kind="ExternalInput")
    out_d = nc.dram_tensor("out", [L, D], FP32, kind="ExternalOutput")

    with tile.TileContext(nc) as tc:
        with ExitStack() as ctx:
            persist = ctx.enter_context(tc.tile_pool(name="persist", bufs=1))
            qT = persist.tile([128, L], FP32, tag="qT")
            kT = persist.tile([128, L], FP32, tag="kT")
            vt = persist.tile([128, 16, 128], FP16, tag="vt")   # [j, jt, 2h*64]
            woT = persist.tile([128, D], FP32, tag="woT")
            woT_h = persist.tile([128, D], FP16, tag="woTh")
            mneg = persist.tile([128, 128], FP32, tag="mneg")
            m01 = persist.tile([128, 128], FP32, tag="m01")
            ident_h = persist.tile([128, 128], FP16, tag="identh")
            zeros_bf = persist.tile([128, L], BF16, tag="zbf")
            trash_a = persist.tile([128, L], BF16, tag="tra")
            trash_f = persist.tile([128, L], BF16, tag="trf")
            trash_dc = persist.tile([128, L], BF16, tag="trdc")

            NST = 32

            def stat(tag):
                return persist.tile([128, NST], FP32, tag=tag, name=tag)

            sum0, sum1, sum2, sum3 = stat("sum0"), stat("sum1"), stat("sum2"), stat("sum3")
            sumD = stat("sumD")
            sm = stat("sm")
            m8a = persist.tile([128, 8, 8], FP32, tag="m8a")
            m8b = persist.tile([128, 8, 8], FP32, tag="m8b")
            invk = persist.tile([128, 8, 8], FP32, tag="invk")
            t0g = persist.tile([128, 8], FP32, tag="t0g")
            Tt, nT = stat("T"), stat("nT")
            Ft, Ct = stat("F"), stat("C")
            rec, Fm, dlt = stat("rec"), stat("Fm"), stat("dlt")
            tau, ntau = stat("tau"), stat("ntau")
            sump, rz = stat("sump"), stat("rz")

            nc.sync.dma_start(mneg[:], mneg_d.ap())
            nc.sync.dma_start(m01[:], m01_d.ap())
            nc.sync.dma_start(woT[:], wo_d.ap())
            ident32 = persist.tile([128, 128], FP32, tag="id32")
            nc.sync.dma_start(ident32[:], ident_d.ap())
            nc.scalar.copy(ident_h[:], ident32[:])
            nc.gpsimd.tensor_copy(woT_h[:], woT[:])
            nc.vector.memset(zeros_bf[:], 0.0)
            for kk in range(8):
                nc.vector.memset(invk[:, :, kk], 1.0 / (kk + 1))
            for s in (sum0, sum1, sum2, sum3, sumD):
                nc.vector.memset(s[:], 0.0)

            # ---------- phase 1: projections ----------
            with ExitStack() as p1:
                ph1 = p1.enter_context(tc.tile_pool(name="ph1", bufs=1))
                ph1p = p1.enter_context(
                    tc.tile_pool(name="ph1p", bufs=2, space="PSUM"))
                xt = ph1.tile([128, 8, L], FP32, tag="xt")
                xt16 = ph1.tile([128, 8, L], FP16, tag="xt16")
                wqs = ph1.tile([128, 8, 128], FP32, tag="wqs")
                wks = ph1.tile([128, 8, 128], FP32, tag="wks")
                wvs = ph1.tile([128, 8, 128], FP32, tag="wvs")
                wvs16 = ph1.tile([128, 8, 128], FP16, tag="wvs16")
                vTs = ph1.tile([128, 512], FP16, tag="vTs")

                xview = xT_d.ap().rearrange("(c p) n -> p c n", p=128)
                for c in range(8):
                    nc.sync.dma_start(xt[:, c, :], xview[:, c, :])
                nc.sync.dma_start(wqs[:], wq_d.ap().rearrange("(c p) m -> p c m", p=128))
                nc.sync.dma_start(wks[:], wk_d.ap().rearrange("(c p) m -> p c m", p=128))
                nc.sync.dma_start(wvs[:], wv_d.ap().rearrange("(c p) m -> p c m", p=128))

                # fp16 copies for the v path
                for c in range(8):
                    if c % 2 == 0:
                        nc.vector.tensor_copy(xt16[:, c, :], xt[:, c, :])
                    else:
                        nc.scalar.copy(xt16[:, c, :], xt[:, c, :])
                nc.gpsimd.tensor_copy(wvs16[:], wvs[:])

                cp_rot = [0]

                def rot_copy(dst, src):
                    if cp_rot[0] % 2 == 0:
                        nc.vector.tensor_copy(dst, src)
                    else:
                        nc.scalar.copy(dst, src)
                    cp_rot[0] += 1

                for dst, wsb in ((qT, wqs), (kT, wks)):
                    for ic in range(4):
                        ps = ph1p.tile([128, 512], FP32, tag="pp")
                        for e in range(8):
                            nc.tensor.matmul(
                                ps[:], wsb[:, e, :],
                                xt[:, e, 512 * ic:512 * (ic + 1)],
                                start=(e == 0), stop=(e == 7))
                        rot_copy(dst[:, 512 * ic:512 * (ic + 1)], ps[:])

                # v via fp16 vT then PE-transpose to [j, d]
                for ic in range(4):
                    ps = ph1p.tile([128, 512], FP32, tag="pp")
                    for e in range(8):
                        nc.tensor.matmul(
                            ps[:], wvs16[:, e, :],
                            xt16[:, e, 512 * ic:512 * (ic + 1)],
                            start=(e == 0), stop=(e == 7))
                    rot_copy(vTs[:], ps[:])
                    nc.sync.dma_start_transpose(
                        vt[:, 4 * ic:4 * ic + 4, :], vTs[:])

            # ---------- phase 2 pools ----------
            s_pool = ctx.enter_context(tc.tile_pool(name="spair", bufs=2))
            p_pool = ctx.enter_context(tc.tile_pool(name="ppair", bufs=1))
            pt_pool = ctx.enter_context(tc.tile_pool(name="ptpair", bufs=1))
            ps_sc = ctx.enter_context(tc.tile_pool(name="ps_sc", bufs=6, space="PSUM"))
            ps_av = ctx.enter_context(tc.tile_pool(name="ps_av", bufs=2, space="PSUM"))
            oc_pool = ctx.enter_context(tc.tile_pool(name="oc", bufs=2))
            otb_pool = ctx.enter_context(tc.tile_pool(name="otb", bufs=2))
            wo_pool = ctx.enter_context(tc.tile_pool(name="woout", bufs=2))

            for g in range(n_groups):
                units = _units_of_group(g)
                gsl = slice(8 * g, 8 * g + 8)
                Sg = [s_pool.tile([128, PAIR_W], FP32, tag=f"sp{s}", name=f"sp{s}_{g}")
                      for s in range(4)]

                # ---- A: scores -> S fp32 masked; sums; maxes ----
                for ui, (rb, h, slot, off) in enumerate(units):
                    col = 8 * g + ui
                    n = 128 * (rb + 1)
                    full = n - 128
                    S = Sg[slot]
                    qw = qT[64 * h:64 * h + 64, 128 * rb:128 * rb + 128]
                    for ci, c0 in enumerate(range(0, n, 512)):
                        w = min(512, n - c0)
                        ps = ps_sc.tile([128, 512], FP32, tag="sc",
                                        name=f"sc{g}_{ui}_{ci}")
                        nc.tensor.matmul(
                            ps[:, :w], qw, kT[64 * h:64 * h + 64, c0:c0 + w],
                            start=True, stop=True)
                        w_nd = min(w, max(0, full - c0))
                        if w_nd > 0:
                            if ui % 2 == 0:
                                nc.scalar.activation(
                                    S[:, off + c0:off + c0 + w_nd], ps[:, :w_nd],
                                    Act.Identity, bias=0.0,
                                    accum_out=(sum0, sum1, sum2, sum3)[ci][:, col:col + 1])
                            else:
                                nc.vector.tensor_scalar(
                                    out=S[:, off + c0:off + c0 + w_nd],
                                    in0=ps[:, :w_nd], scalar1=0.0, scalar2=0.0,
                                    op0=Alu.add, op1=Alu.add,
                                    accum_out=(sum0, sum1, sum2, sum3)[ci][:, col:col + 1])
                        if c0 + w > full:
                            ld = full - c0
                            nc.vector.tensor_tensor(
                                S[:, off + full:off + n], ps[:, ld:ld + 128],
                                mneg[:], Alu.add)
                            nc.vector.scalar_tensor_tensor(
                                out=trash_dc[:, :128],
                                in0=ps[:, ld:ld + 128], scalar=1.0, in1=m01[:],
                                op0=Alu.mult, op1=Alu.mult,
                                accum_out=sumD[:, col:col + 1])
                    # top-8 values per row (sorted desc) for the tau seed
                    nc.vector.max(m8a[:, ui, :], S[:, off:off + n])

                nc.vector.tensor_tensor(Fm[:, gsl], sum0[:, gsl], sum1[:, gsl], Alu.add)
                nc.vector.tensor_tensor(dlt[:, gsl], sum2[:, gsl], sum3[:, gsl], Alu.add)
                nc.vector.tensor_tensor(sm[:, gsl], Fm[:, gsl], dlt[:, gsl], Alu.add)
                nc.vector.tensor_tensor(sm[:, gsl], sm[:, gsl], sumD[:, gsl], Alu.add)
                # T0 = max_k (cumsum_k(top8) - 1)/k  (running-tau peak)
                nc.vector.tensor_copy(m8b[:], m8a[:])
                nc.vector.tensor_tensor(
                    m8b[:, :, 1:8], m8a[:, :, 1:8], m8a[:, :, 0:7], Alu.add)
                nc.vector.tensor_copy(m8a[:], m8b[:])
                nc.vector.tensor_tensor(
                    m8a[:, :, 2:8], m8b[:, :, 2:8], m8b[:, :, 0:6], Alu.add)
                nc.vector.tensor_copy(m8b[:], m8a[:])
                nc.vector.tensor_tensor(
                    m8b[:, :, 4:8], m8a[:, :, 4:8], m8a[:, :, 0:4], Alu.add)
                nc.vector.tensor_scalar_add(m8b[:], m8b[:], -1.0)
                nc.vector.tensor_tensor(m8b[:], m8b[:], invk[:], Alu.mult)
                nc.vector.tensor_reduce(t0g[:], m8b[:], mybir.AxisListType.X,
                                        Alu.max)
                nc.vector.tensor_copy(Tt[:, gsl], t0g[:])
                nc.vector.tensor_scalar_mul(nT[:, gsl], t0g[:], -1.0)

                # ---- Newton iterations ----
                def emit_F(ui, rb, h, slot, off):
                    col = 8 * g + ui
                    n = 128 * (rb + 1)
                    S = Sg[slot]
                    if ui < 6:
                        nc.scalar.activation(
                            trash_a[:, :n], S[:, off:off + n], Act.Relu,
                            bias=nT[:, col:col + 1],
                            accum_out=Ft[:, col:col + 1])
                    else:
                        nc.vector.scalar_tensor_tensor(
                            out=trash_f[:, :n], in0=S[:, off:off + n],
                            scalar=nT[:, col:col + 1], in1=zeros_bf[:, :n],
                            op0=Alu.add, op1=Alu.max,
                            accum_out=Ft[:, col:col + 1])

                def emit_cnt(ui, rb, h, slot, off):
                    col = 8 * g + ui
                    n = 128 * (rb + 1)
                    S = Sg[slot]
                    nc.vector.tensor_scalar(
                        out=trash_dc[:, :n], in0=S[:, off:off + n],
                        scalar1=Tt[:, col:col + 1], scalar2=0.0,
                        op0=Alu.is_gt, op1=Alu.add,
                        accum_out=Ct[:, col:col + 1])

                for it in range(N_ITERS):
                    for ui, (rb, h, slot, off) in enumerate(units):
                        emit_F(ui, rb, h, slot, off)
                        emit_cnt(ui, rb, h, slot, off)
                    nc.vector.tensor_scalar_max(Ct[:, gsl], Ct[:, gsl], 1.0)
                    nc.vector.reciprocal(rec[:, gsl], Ct[:, gsl])
                    nc.vector.tensor_scalar_add(Fm[:, gsl], Ft[:, gsl], -1.0)
                    nc.vector.tensor_tensor(dlt[:, gsl], Fm[:, gsl], rec[:, gsl], Alu.mult)
                    nc.vector.tensor_tensor(Tt[:, gsl], Tt[:, gsl], dlt[:, gsl], Alu.add)
                    nc.vector.tensor_tensor(nT[:, gsl], nT[:, gsl], dlt[:, gsl], Alu.subtract)

                # ---- final count; tau_star ----
                for ui, (rb, h, slot, off) in enumerate(units):
                    emit_cnt(ui, rb, h, slot, off)
                nc.vector.tensor_scalar_max(Ct[:, gsl], Ct[:, gsl], 1.0)
                nc.vector.reciprocal(rec[:, gsl], Ct[:, gsl])
                nc.vector.tensor_scalar_add(Fm[:, gsl], sm[:, gsl], -1.0)
                nc.vector.tensor_tensor(tau[:, gsl], Fm[:, gsl], rec[:, gsl], Alu.mult)
                nc.vector.tensor_scalar_mul(ntau[:, gsl], tau[:, gsl], -1.0)

                # ---- P = relu(S - tau) fp16; Pt = P^T via DMA xbar; AV; Wo ----
                Pg = [p_pool.tile([128, PAIR_W], FP16, tag=f"pp{s}", name=f"pp{s}_{g}")
                      for s in range(4)]
                Ptg = [pt_pool.tile([128, 17, 128], FP16, tag=f"pt{s}",
                                    name=f"pt{s}_{g}")
                       for s in range(4)]
                for ui, (rb, h, slot, off) in enumerate(units):
                    col = 8 * g + ui
                    n = 128 * (rb + 1)
                    S, P = Sg[slot], Pg[slot]
                    if ui < 4:
                        nc.scalar.activation(
                            P[:, off:off + n], S[:, off:off + n], Act.Relu,
                            bias=ntau[:, col:col + 1],
                            accum_out=sump[:, col:col + 1])
                    else:
                        nc.vector.tensor_scalar(
                            out=P[:, off:off + n], in0=S[:, off:off + n],
                            scalar1=ntau[:, col:col + 1], scalar2=0.0,
                            op0=Alu.add, op1=Alu.max)
                        nc.vector.tensor_scalar(
                            out=trash_f[:, :n], in0=P[:, off:off + n],
                            scalar1=0.0, scalar2=0.0,
                            op0=Alu.add, op1=Alu.add,
                            accum_out=sump[:, col:col + 1])
                    nc.vector.tensor_scalar_add(
                        Fm[:, col:col + 1], sump[:, col:col + 1], 1.0e-10)
                    nc.vector.reciprocal(rz[:, col:col + 1], Fm[:, col:col + 1])

                # one xbar transpose per pair-slot
                for slot in range(4):
                    nc.sync.dma_start_transpose(Ptg[slot][:], Pg[slot][:])

                outc_of_rb = {}
                for ui, (rb, h, slot, off) in enumerate(units):
                    col = 8 * g + ui
                    n = 128 * (rb + 1)
                    Pt = Ptg[slot]
                    bt0 = off // 128
                    if rb not in outc_of_rb:
                        outc_of_rb[rb] = oc_pool.tile(
                            [128, 128], FP16, tag=f"oc{ui % 2}", name=f"oc{g}_{rb}")
                    outc = outc_of_rb[rb]

                    av = ps_av.tile([128, 512], FP32, tag="av", name=f"av{g}_{ui}")
                    nt = n // 128
                    for jt in range(nt):
                        nc.tensor.matmul(
                            av[:, :64], Pt[:, bt0 + jt, :],
                            vt[:, jt, 64 * h:64 * h + 64],
                            start=(jt == 0), stop=(jt == nt - 1))
                    nc.scalar.activation(
                        outc[:, 64 * h:64 * h + 64], av[:, :64], Act.Copy,
                        bias=0.0, scale=rz[:, col:col + 1])

                    if h == 1:
                        wo_out = wo_pool.tile([128, D], FP32, tag="wod",
                                              name=f"wod{g}_{rb}")
                        otb = otb_pool.tile([128, 128], FP16, tag="otbh",
                                            name=f"otb{g}_{rb}")
                        nc.sync.dma_start_transpose(otb[:], outc[:])
                        for oc2 in range(2):
                            wps = ps_av.tile([128, 512], FP32, tag="av",
                                             name=f"wo{g}_{rb}_{oc2}")
                            nc.tensor.matmul(
                                wps[:], otb[:],
                                woT_h[:, 512 * oc2:512 * (oc2 + 1)],
                                start=True, stop=True)
                            if oc2 == 0:
                                nc.scalar.copy(wo_out[:, :512], wps[:])
                            else:
                                nc.vector.tensor_copy(wo_out[:, 512:], wps[:])
                        nc.sync.dma_start(
                            out_d.ap()[128 * rb:128 * (rb + 1), :], wo_out[:])

    nc.compile()
    return nc


_CACHE = {}


def _get_nc():
    if "nc" not in _CACHE:
        _CACHE["nc"] = build_program()
    return _CACHE["nc"]


def _host_inputs(x, Wq, Wk, Wv, Wo):
    xT = np.ascontiguousarray(x[0].T).astype(np.float32)
    ii = np.arange(128)
    mneg = np.where(ii[None, :] > ii[:, None], np.float32(NEG_BIG),
                    np.float32(0.0)).astype(np.float32)
    m01 = (ii[None, :] <= ii[:, None]).astype(np.float32)
    in_maps = []
    for c in range(N_CORES):
        hsl = slice(128 * c, 128 * (c + 1))
        in_maps.append({
            "xT": xT,
            "wqT": np.ascontiguousarray((Wq[hsl] * np.float32(SCALE)).T).astype(np.float32),
            "wkT": np.ascontiguousarray(Wk[hsl].T).astype(np.float32),
            "wvT": np.ascontiguousarray(Wv[hsl].T).astype(np.float32),
            "woT": np.ascontiguousarray(Wo[:, hsl].T).astype(np.float32),
            "mneg": mneg,
            "m01": m01,
            "ident": np.eye(128, dtype=np.float32),
        })
    return in_maps


def kernel(x, Wq, Wk, Wv, Wo, _trace=False):
    nc = _get_nc()
    in_maps = _host_inputs(np.asarray(x), np.asarray(Wq), np.asarray(Wk),
                           np.asarray(Wv), np.asarray(Wo))
    res = run_bass_kernel_spmd(nc, in_maps, core_ids=list(range(N_CORES)),
                               trace=_trace)
    out = np.zeros((L, D), np.float32)
    for c in range(N_CORES):
        out += res.results[c]["out"]
    if _trace:
        _CACHE["last_results"] = res
    return out.reshape(1, L, D)


# revision 5
# speedup vs baseline: 1.9467x; 1.1232x over previous
"""Entmax attention Trainium2 kernel v5 (8-core SPMD, head-parallel).

HW-calibrated design (v4 trace: DVE 86%, PE 76% w/ 365us of gaps, ACT 43%):
- top-8 seed T0 = max_k (cumsum_k-1)/k, 3 plain Newton iters (100% exact
  k_support at 3 in numpy), final fp32 count.
- Per-iteration engine split calibrated to HW (everything ~1 elem/cycle):
  F: units 0-4 ACT Relu+accum, 5-7 DVE stt; cnt: units 0-4 DVE is_gt+accum,
  5-7 ACT Sign+accum with (A + n)/2 fixup (masked entries count as -1).
- Pipelined emission: phaseA(g+1) is emitted before the AV/Wo tail of g so
  the PE's in-order queue can run scores g+1 and AV g during Newton phases.
- AV batched jt-major: P^T DMA-xbar-transposed into a zero-padded
  [128, 16jt, 4slot, 128] layout per head; one matmul per (head, jt) with a
  512-wide moving operand accumulating all 4 row-blocks at once
  (68 matmuls+ldweights per group -> ~29).
"""
import numpy as np
from contextlib import ExitStack

import concourse.bass as bass
import concourse.tile as tile
import concourse.mybir as mybir
from concourse import bacc
from concourse.bass_utils import run_bass_kernel_spmd

L = 2048
D = 1024
H = 16
HD = 64
N_CORES = 8
HPC = 2
SCALE = float(HD) ** -0.5

FP32 = mybir.dt.float32
FP16 = mybir.dt.float16
BF16 = mybir.dt.bfloat16
Alu = mybir.AluOpType
Act = mybir.ActivationFunctionType

N_ITERS = 3
NEG_BIG = -1.0e30
MAX_INIT = -3.0e38

RB_PAIRS = [(0, 15), (4, 11), (1, 14), (5, 10), (2, 13), (6, 9), (3, 12), (7, 8)]
PAIR_W = 17 * 128  # 2176


def _units_of_group(g):
    units = []
    p0, p1 = RB_PAIRS[2 * g], RB_PAIRS[2 * g + 1]
    for h in range(HPC):
        for pi_local, (ra, rb_) in enumerate((p0, p1)):
            slot = 2 * pi_local + h
            na = 128 * (ra + 1)
            units.append((ra, h, slot, 0))
            units.append((rb_, h, slot, na))
    return units


def build_program(n_groups=4):
    nc = bacc.Bacc("TRN2", target_bir_lowering=False, debug=False, num_devices=1)

    xT_d = nc.dram_tensor("xT", [D, L], FP32, kind="ExternalInput")
    wq_d = nc.dram_tensor("wqT", [D, 128], FP32, kind="ExternalInput")
    wk_d = nc.dram_tensor("wkT", [D, 128], FP32, kind="ExternalInput")
    wv_d = nc.dram_tensor("wvT", [D, 128], FP32, kind="ExternalInput")
    wo_d = nc.dram_tensor("woT", [128, D], FP32, kind="ExternalInput")
    mneg_d = nc.dram_tensor("mneg", [128, 128], FP32, kind="ExternalInput")
    m01_d = nc.dram_tensor("m01", [128, 128], FP32, kind="ExternalInput")
    ident_d = nc.dram_tensor("ident", [128, 128], FP32, kind="ExternalInput")
    out_d = nc.dram_tensor("out", [L, D], FP32, kind="ExternalOutput")

    with tile.TileContext(nc) as tc:
        with ExitStack() as ctx:
            persist = ctx.enter_context(tc.tile_pool(name="persist", bufs=1))
            qT = persist.tile([128, L], FP32, tag="qT")
            kT = persist.tile([128, L], FP32, tag="kT")
            vt = persist.tile([128, 16, 128], FP16, tag="vt")
            woT = persist.tile([128, D], FP32, tag="woT")
            woT_h = persist.tile([128, D], FP16, tag="woTh")
            mneg = persist.tile([128, 128], FP32, tag="mneg")
            m01 = persist.tile([128, 128], FP32, tag="m01")
            zeros_bf = persist.tile([128, L], BF16, tag="zbf")
            trash_a = persist.tile([128, L], BF16, tag="tra")
            trash_f = persist.tile([128, L], BF16, tag="trf")
            trash_dc = persist.tile([128, L], BF16, tag="trdc")
            # zero-padded transposed-P, one per head: [j, jt, slot, i]
            pth = [persist.tile([128, 16, 4, 128], FP16, tag=f"pth{h}",
                                name=f"pth{h}")
                   for h in range(HPC)]

            NST = 32

            def stat(tag):
                return persist.tile([128, NST], FP32, tag=tag, name=tag)

            sum0, sum1, sum2, sum3 = stat("sum0"), stat("sum1"), stat("sum2"), stat("sum3")
            sumD = stat("sumD")
            sm = stat("sm")
            nh = stat("nh")
            Tt, nT = stat("T"), stat("nT")
            Ft, Ct = stat("F"), stat("C")
            rec, Fm, dlt = stat("rec"), stat("Fm"), stat("dlt")
            tau, ntau = stat("tau"), stat("ntau")
            sump, rz = stat("sump"), stat("rz")
            m8a = persist.tile([128, 8, 8], FP32, tag="m8a")
            m8b = persist.tile([128, 8, 8], FP32, tag="m8b")
            invk = persist.tile([128, 8, 8], FP32, tag="invk")
            t0g = persist.tile([128, 8], FP32, tag="t0g")

            nc.sync.dma_start(mneg[:], mneg_d.ap())
            nc.sync.dma_start(m01[:], m01_d.ap())
            nc.sync.dma_start(woT[:], wo_d.ap())
            nc.gpsimd.tensor_copy(woT_h[:], woT[:])
            nc.vector.memset(zeros_bf[:], 0.0)
            for s in (sum0, sum1, sum2, sum3, sumD):
                nc.vector.memset(s[:], 0.0)
            for kk in range(8):
                nc.vector.memset(invk[:, :, kk], 1.0 / (kk + 1))
            for g in range(n_groups):
                for ui, (rb, h, slot, off) in enumerate(_units_of_group(g)):
                    nc.vector.memset(nh[:, 8 * g + ui:8 * g + ui + 1],
                                     64.0 * (rb + 1))
            for h in range(HPC):
                nc.vector.memset(pth[h][:], 0.0)

            # ---------- phase 1: projections ----------
            with ExitStack() as p1:
                ph1 = p1.enter_context(tc.tile_pool(name="ph1", bufs=1))
                ph1p = p1.enter_context(
                    tc.tile_pool(name="ph1p", bufs=2, space="PSUM"))
                xt = ph1.tile([128, 8, L], FP32, tag="xt")
                xt16 = ph1.tile([128, 8, L], FP16, tag="xt16")
                wqs = ph1.tile([128, 8, 128], FP32, tag="wqs")
                wks = ph1.tile([128, 8, 128], FP32, tag="wks")
                wvs = ph1.tile([128, 8, 128], FP32, tag="wvs")
                wvs16 = ph1.tile([128, 8, 128], FP16, tag="wvs16")
                vTs = ph1.tile([128, 512], FP16, tag="vTs")

                xview = xT_d.ap().rearrange("(c p) n -> p c n", p=128)
                for c in range(8):
                    nc.sync.dma_start(xt[:, c, :], xview[:, c, :])
                nc.sync.dma_start(wqs[:], wq_d.ap().rearrange("(c p) m -> p c m", p=128))
                nc.sync.dma_start(wks[:], wk_d.ap().rearrange("(c p) m -> p c m", p=128))
                nc.sync.dma_start(wvs[:], wv_d.ap().rearrange("(c p) m -> p c m", p=128))

                for c in range(8):
                    if c % 2 == 0:
                        nc.vector.tensor_copy(xt16[:, c, :], xt[:, c, :])
                    else:
                        nc.scalar.copy(xt16[:, c, :], xt[:, c, :])
                nc.gpsimd.tensor_copy(wvs16[:], wvs[:])

                cp_rot = [0]

                def rot_copy(dst, src):
                    if cp_rot[0] % 2 == 0:
                        nc.vector.tensor_copy(dst, src)
                    else:
                        nc.scalar.copy(dst, src)
                    cp_rot[0] += 1

                for dst, wsb in ((qT, wqs), (kT, wks)):
                    for ic in range(4):
                        ps = ph1p.tile([128, 512], FP32, tag="pp")
                        for e in range(8):
                            nc.tensor.matmul(
                                ps[:], wsb[:, e, :],
                                xt[:, e, 512 * ic:512 * (ic + 1)],
                                start=(e == 0), stop=(e == 7))
                        rot_copy(dst[:, 512 * ic:512 * (ic + 1)], ps[:])

                for ic in range(4):
                    ps = ph1p.tile([128, 512], FP32, tag="pp")
                    for e in range(8):
                        nc.tensor.matmul(
                            ps[:], wvs16[:, e, :],
                            xt16[:, e, 512 * ic:512 * (ic + 1)],
                            start=(e == 0), stop=(e == 7))
                    rot_copy(vTs[:], ps[:])
                    nc.sync.dma_start_transpose(
                        vt[:, 4 * ic:4 * ic + 4, :], vTs[:])

            # ---------- phase 2 pools ----------
            s_pool = ctx.enter_context(tc.tile_pool(name="spair", bufs=2))
            p_pool = ctx.enter_context(tc.tile_pool(name="ppair", bufs=2))
            ps_sc = ctx.enter_context(tc.tile_pool(name="ps_sc", bufs=5, space="PSUM"))
            ps_av = ctx.enter_context(tc.tile_pool(name="ps_av", bufs=3, space="PSUM"))
            avh_pool = ctx.enter_context(tc.tile_pool(name="avh", bufs=2))
            avt_pool = ctx.enter_context(tc.tile_pool(name="avt", bufs=4))
            oc_pool = ctx.enter_context(tc.tile_pool(name="oc", bufs=2))
            otb_pool = ctx.enter_context(tc.tile_pool(name="otb", bufs=2))
            wo_pool = ctx.enter_context(tc.tile_pool(name="woout", bufs=2))

            Sg_of = {}

            def emit_phaseA(g):
                units = _units_of_group(g)
                gsl = slice(8 * g, 8 * g + 8)
                Sg = [s_pool.tile([128, PAIR_W], FP32, tag=f"sp{s}",
                                  name=f"sp{s}_{g}") for s in range(4)]
                Sg_of[g] = Sg
                chunk_ctr = [0]
                for ui, (rb, h, slot, off) in enumerate(units):
                    col = 8 * g + ui
                    n = 128 * (rb + 1)
                    full = n - 128
                    S = Sg[slot]
                    qw = qT[64 * h:64 * h + 64, 128 * rb:128 * rb + 128]
                    for ci, c0 in enumerate(range(0, n, 512)):
                        w = min(512, n - c0)
                        ps = ps_sc.tile([128, 512], FP32, tag="sc",
                                        name=f"sc{g}_{ui}_{ci}")
                        nc.tensor.matmul(
                            ps[:, :w], qw, kT[64 * h:64 * h + 64, c0:c0 + w],
                            start=True, stop=True)
                        w_nd = min(w, max(0, full - c0))
                        if w_nd > 0:
                            acc = (sum0, sum1, sum2, sum3)[ci][:, col:col + 1]
                            if chunk_ctr[0] % 3 != 2:
                                nc.scalar.activation(
                                    S[:, off + c0:off + c0 + w_nd], ps[:, :w_nd],
                                    Act.Identity, bias=0.0, accum_out=acc)
                            else:
                                nc.vector.tensor_scalar(
                                    out=S[:, off + c0:off + c0 + w_nd],
                                    in0=ps[:, :w_nd], scalar1=0.0, scalar2=0.0,
                                    op0=Alu.add, op1=Alu.add, accum_out=acc)
                            chunk_ctr[0] += 1
                        if c0 + w > full:
                            ld = full - c0
                            nc.vector.tensor_tensor(
                                S[:, off + full:off + n], ps[:, ld:ld + 128],
                                mneg[:], Alu.add)
                            nc.vector.scalar_tensor_tensor(
                                out=trash_dc[:, :128],
                                in0=ps[:, ld:ld + 128], scalar=1.0, in1=m01[:],
                                op0=Alu.mult, op1=Alu.mult,
                                accum_out=sumD[:, col:col + 1])
                    nc.vector.max(m8a[:, ui, :], S[:, off:off + n])

                # row sums; top8 -> T0
                nc.vector.tensor_tensor(Fm[:, gsl], sum0[:, gsl], sum1[:, gsl], Alu.add)
                nc.vector.tensor_tensor(dlt[:, gsl], sum2[:, gsl], sum3[:, gsl], Alu.add)
                nc.vector.tensor_tensor(sm[:, gsl], Fm[:, gsl], dlt[:, gsl], Alu.add)
                nc.vector.tensor_tensor(sm[:, gsl], sm[:, gsl], sumD[:, gsl], Alu.add)
                nc.vector.tensor_copy(m8b[:], m8a[:])
                nc.vector.tensor_tensor(
                    m8b[:, :, 1:8], m8a[:, :, 1:8], m8a[:, :, 0:7], Alu.add)
                nc.vector.tensor_copy(m8a[:], m8b[:])
                nc.vector.tensor_tensor(
                    m8a[:, :, 2:8], m8b[:, :, 2:8], m8b[:, :, 0:6], Alu.add)
                nc.vector.tensor_copy(m8b[:], m8a[:])
                nc.vector.tensor_tensor(
                    m8b[:, :, 4:8], m8a[:, :, 4:8], m8a[:, :, 0:4], Alu.add)
                nc.vector.tensor_scalar_add(m8b[:], m8b[:], -1.0)
                nc.vector.tensor_tensor(m8b[:], m8b[:], invk[:], Alu.mult)
                nc.vector.tensor_reduce(t0g[:], m8b[:], mybir.AxisListType.X,
                                        Alu.max)
                nc.vector.tensor_copy(Tt[:, gsl], t0g[:])
                nc.vector.tensor_scalar_mul(nT[:, gsl], t0g[:], -1.0)

            def emit_F(g, ui, rb, h, slot, off):
                col = 8 * g + ui
                n = 128 * (rb + 1)
                S = Sg_of[g][slot]
                if ui < 5:
                    nc.scalar.activation(
                        trash_a[:, :n], S[:, off:off + n], Act.Relu,
                        bias=nT[:, col:col + 1],
                        accum_out=Ft[:, col:col + 1])
                else:
                    nc.vector.scalar_tensor_tensor(
                        out=trash_f[:, :n], in0=S[:, off:off + n],
                        scalar=nT[:, col:col + 1], in1=zeros_bf[:, :n],
                        op0=Alu.add, op1=Alu.max,
                        accum_out=Ft[:, col:col + 1])

            def emit_cnt(g, ui, rb, h, slot, off):
                col = 8 * g + ui
                n = 128 * (rb + 1)
                S = Sg_of[g][slot]
                if ui < 5:
                    nc.vector.tensor_scalar(
                        out=trash_dc[:, :n], in0=S[:, off:off + n],
                        scalar1=Tt[:, col:col + 1], scalar2=0.0,
                        op0=Alu.is_gt, op1=Alu.add,
                        accum_out=Ct[:, col:col + 1])
                else:
                    # Sign trick: masked (-1e30) entries count -1, so
                    # cnt = 0.5*A + n/2 with n the full padded width.
                    nc.scalar.activation(
                        trash_a[:, :n], S[:, off:off + n], Act.Sign,
                        bias=nT[:, col:col + 1],
                        accum_out=Ct[:, col:col + 1])

            def cnt_fixup(g):
                hsl = slice(8 * g + 5, 8 * g + 8)
                nc.vector.scalar_tensor_tensor(
                    out=Ct[:, hsl], in0=Ct[:, hsl], scalar=0.5, in1=nh[:, hsl],
                    op0=Alu.mult, op1=Alu.add)

            def emit_newton(g):
                units = _units_of_group(g)
                gsl = slice(8 * g, 8 * g + 8)
                for it in range(N_ITERS):
                    for ui, (rb, h, slot, off) in enumerate(units):
                        emit_F(g, ui, rb, h, slot, off)
                        emit_cnt(g, ui, rb, h, slot, off)
                    cnt_fixup(g)
                    nc.vector.tensor_scalar_max(Ct[:, gsl], Ct[:, gsl], 1.0)
                    nc.vector.reciprocal(rec[:, gsl], Ct[:, gsl])
                    nc.vector.tensor_scalar_add(Fm[:, gsl], Ft[:, gsl], -1.0)
                    nc.vector.tensor_tensor(dlt[:, gsl], Fm[:, gsl], rec[:, gsl], Alu.mult)
                    nc.vector.tensor_tensor(Tt[:, gsl], Tt[:, gsl], dlt[:, gsl], Alu.add)
                    nc.vector.tensor_tensor(nT[:, gsl], nT[:, gsl], dlt[:, gsl], Alu.subtract)
                # final count
                for ui, (rb, h, slot, off) in enumerate(units):
                    emit_cnt(g, ui, rb, h, slot, off)
                cnt_fixup(g)
                nc.vector.tensor_scalar_max(Ct[:, gsl], Ct[:, gsl], 1.0)
                nc.vector.reciprocal(rec[:, gsl], Ct[:, gsl])
                nc.vector.tensor_scalar_add(Fm[:, gsl], sm[:, gsl], -1.0)
                nc.vector.tensor_tensor(tau[:, gsl], Fm[:, gsl], rec[:, gsl], Alu.mult)
                nc.vector.tensor_scalar_mul(ntau[:, gsl], tau[:, gsl], -1.0)

            def emit_P(g):
                units = _units_of_group(g)
                Pg = [p_pool.tile([128, PAIR_W], FP16, tag=f"pp{s}",
                                  name=f"pp{s}_{g}") for s in range(4)]
                for ui, (rb, h, slot, off) in enumerate(units):
                    col = 8 * g + ui
                    n = 128 * (rb + 1)
                    S, P = Sg_of[g][slot], Pg[slot]
                    if ui < 6:
                        nc.scalar.activation(
                            P[:, off:off + n], S[:, off:off + n], Act.Relu,
                            bias=ntau[:, col:col + 1],
                            accum_out=sump[:, col:col + 1])
                    else:
                        nc.vector.tensor_scalar(
                            out=P[:, off:off + n], in0=S[:, off:off + n],
                            scalar1=ntau[:, col:col + 1], scalar2=0.0,
                            op0=Alu.add, op1=Alu.max)
                        nc.vector.tensor_scalar(
                            out=trash_f[:, :n], in0=P[:, off:off + n],
                            scalar1=0.0, scalar2=0.0,
                            op0=Alu.add, op1=Alu.add,
                            accum_out=sump[:, col:col + 1])
                    nc.vector.tensor_scalar_add(
                        Fm[:, col:col + 1], sump[:, col:col + 1], 1.0e-10)
                    nc.vector.reciprocal(rz[:, col:col + 1], Fm[:, col:col + 1])
                return Pg

            def emit_tail(g, Pg):
                units = _units_of_group(g)
                # slot -> nt for this group; stale-block zeroing for shrinking slots
                nts = {}
                for ui, (rb, h, slot, off) in enumerate(units):
                    if h == 0:
                        nts[slot // 2 * 2 + (1 if off > 0 else 0)] = rb + 1
                # slots in pth layout: index by (pair_local, a/b) = 0..3
                # unit slot s in Sg corresponds to pth slot: derive from units
                # pth slot assignment: use (pi_local*2 + is_b)
                pth_slot = {}
                for ui, (rb, h, slot, off) in enumerate(units):
                    pi_local = slot // 2
                    is_b = 1 if off > 0 else 0
                    pth_slot[ui] = pi_local * 2 + is_b

                if g > 0:
                    prev = _units_of_group(g - 1)
                    for ui, (rb, h, slot, off) in enumerate(units):
                        psl = pth_slot[ui]
                        (prb, _, _, poff) = prev[ui]
                        nt, pnt = rb + 1, prb + 1
                        if nt < pnt:
                            # zero the now-stale jt blocks
                            nc.vector.memset(
                                pth[h][:, nt:pnt, psl, :], 0.0)

                # P^T via DMA xbar into pth
                for ui, (rb, h, slot, off) in enumerate(units):
                    nt = rb + 1
                    psl = pth_slot[ui]
                    nc.sync.dma_start_transpose(
                        pth[h][:, 0:nt, psl, :],
                        Pg[slot][:, off:off + 128 * nt])

                # AV: one matmul per (head, jt), 512-wide moving
                maxnt = max(rb + 1 for (rb, _, _, _) in units)
                avps = {}
                for h in range(HPC):
                    avps[h] = ps_av.tile([128, 512], FP32, tag="av",
                                         name=f"av{g}_{h}")
                    for jt in range(maxnt):
                        nc.tensor.matmul(
                            avps[h][:64, :],
                            vt[:, jt, 64 * h:64 * h + 64],
                            pth[h][:, jt, :, :],
                            start=(jt == 0), stop=(jt == maxnt - 1))

                # avps [64d, 4slot*128i] -> fp16 -> per-block DMA transpose
                outc_of_rb = {}
                avh = {}
                for h in range(HPC):
                    avh[h] = avh_pool.tile([128, 512], FP16, tag="avh",
                                           name=f"avh{g}_{h}")
                    if h == 0:
                        nc.scalar.copy(avh[h][:64, :], avps[h][:64, :])
                    else:
                        nc.vector.tensor_copy(avh[h][:64, :], avps[h][:64, :])

                for ui, (rb, h, slot, off) in enumerate(units):
                    col = 8 * g + ui
                    psl = pth_slot[ui]
                    avt = avt_pool.tile([128, 64], FP16, tag="avt",
                                        name=f"avt{g}_{ui}")
                    nc.sync.dma_start_transpose(
                        avt[:], avh[h][:64, 128 * psl:128 * (psl + 1)])
                    if rb not in outc_of_rb:
                        outc_of_rb[rb] = oc_pool.tile(
                            [128, 128], FP16, tag=f"oc{ui % 2}",
                            name=f"oc{g}_{rb}")
                    outc = outc_of_rb[rb]
                    nc.scalar.activation(
                        outc[:, 64 * h:64 * h + 64], avt[:], Act.Copy,
                        bias=0.0, scale=rz[:, col:col + 1])

                    if h == 1:
                        wo_out = wo_pool.tile([128, D], FP32, tag="wod",
                                              name=f"wod{g}_{rb}")
                        otb = otb_pool.tile([128, 128], FP16, tag="otbh",
                                            name=f"otb{g}_{rb}")
                        nc.sync.dma_start_transpose(otb[:], outc[:])
                        for oc2 in range(2):
                            wps = ps_av.tile([128, 512], FP32, tag="av",
                                             name=f"wo{g}_{rb}_{oc2}")
                            nc.tensor.matmul(
                                wps[:], otb[:],
                                woT_h[:, 512 * oc2:512 * (oc2 + 1)],
                                start=True, stop=True)
                            if oc2 == 0:
                                nc.scalar.copy(wo_out[:, :512], wps[:])
                            else:
                                nc.vector.tensor_copy(wo_out[:, 512:], wps[:])
                        nc.sync.dma_start(
                            out_d.ap()[128 * rb:128 * (rb + 1), :], wo_out[:])

            # ---------- pipelined emission ----------
            emit_phaseA(0)
            pending = None  # (g, Pg) awaiting tail
            for g in range(n_groups):
                emit_newton(g)
                Pg = emit_P(g)
                if g + 1 < n_groups:
                    emit_phaseA(g + 1)
                emit_tail(g, Pg)

    nc.compile()
    return nc


_CACHE = {}


def _get_nc():
    if "nc" not in _CACHE:
        _CACHE["nc"] = build_program()
    return _CACHE["nc"]


def _host_inputs(x, Wq, Wk, Wv, Wo):
    xT = np.ascontiguousarray(x[0].T).astype(np.float32)
    ii = np.arange(128)
    mneg = np.where(ii[None, :] > ii[:, None], np.float32(NEG_BIG),
                    np.float32(0.0)).astype(np.float32)
    m01 = (ii[None, :] <= ii[:, None]).astype(np.float32)
    in_maps = []
    for c in range(N_CORES):
        hsl = slice(128 * c, 128 * (c + 1))
        in_maps.append({
            "xT": xT,
            "wqT": np.ascontiguousarray((Wq[hsl] * np.float32(SCALE)).T).astype(np.float32),
            "wkT": np.ascontiguousarray(Wk[hsl].T).astype(np.float32),
            "wvT": np.ascontiguousarray(Wv[hsl].T).astype(np.float32),
            "woT": np.ascontiguousarray(Wo[:, hsl].T).astype(np.float32),
            "mneg": mneg,
            "m01": m01,
            "ident": np.eye(128, dtype=np.float32),
        })
    return in_maps


def kernel(x, Wq, Wk, Wv, Wo, _trace=False):
    nc = _get_nc()
    in_maps = _host_inputs(np.asarray(x), np.asarray(Wq), np.asarray(Wk),
                           np.asarray(Wv), np.asarray(Wo))
    res = run_bass_kernel_spmd(nc, in_maps, core_ids=list(range(N_CORES)),
                               trace=_trace)
    out = np.zeros((L, D), np.float32)
    for c in range(N_CORES):
        out += res.results[c]["out"]
    if _trace:
        _CACHE["last_results"] = res
    return out.reshape(1, L, D)


# revision 6
# speedup vs baseline: 2.0504x; 1.0533x over previous
"""Entmax attention Trainium2 kernel v5 (8-core SPMD, head-parallel).

HW-calibrated design (v4 trace: DVE 86%, PE 76% w/ 365us of gaps, ACT 43%):
- top-8 seed T0 = max_k (cumsum_k-1)/k, 3 plain Newton iters (100% exact
  k_support at 3 in numpy), final fp32 count.
- Per-iteration engine split calibrated to HW (everything ~1 elem/cycle):
  F: units 0-4 ACT Relu+accum, 5-7 DVE stt; cnt: units 0-4 DVE is_gt+accum,
  5-7 ACT Sign+accum with (A + n)/2 fixup (masked entries count as -1).
- Pipelined emission: phaseA(g+1) is emitted before the AV/Wo tail of g so
  the PE's in-order queue can run scores g+1 and AV g during Newton phases.
- AV batched jt-major: P^T DMA-xbar-transposed into a zero-padded
  [128, 16jt, 4slot, 128] layout per head; one matmul per (head, jt) with a
  512-wide moving operand accumulating all 4 row-blocks at once
  (68 matmuls+ldweights per group -> ~29).
"""
import numpy as np
from contextlib import ExitStack

import concourse.bass as bass
import concourse.tile as tile
import concourse.mybir as mybir
from concourse import bacc
from concourse.bass_utils import run_bass_kernel_spmd

L = 2048
D = 1024
H = 16
HD = 64
N_CORES = 8
HPC = 2
SCALE = float(HD) ** -0.5

FP32 = mybir.dt.float32
FP16 = mybir.dt.float16
BF16 = mybir.dt.bfloat16
Alu = mybir.AluOpType
Act = mybir.ActivationFunctionType

N_ITERS = 3
NEG_BIG = -1.0e30
MAX_INIT = -3.0e38

RB_PAIRS = [(0, 15), (4, 11), (1, 14), (5, 10), (2, 13), (6, 9), (3, 12), (7, 8)]
PAIR_W = 17 * 128  # 2176


def _units_of_group(g):
    units = []
    p0, p1 = RB_PAIRS[2 * g], RB_PAIRS[2 * g + 1]
    for h in range(HPC):
        for pi_local, (ra, rb_) in enumerate((p0, p1)):
            slot = 2 * pi_local + h
            na = 128 * (ra + 1)
            units.append((ra, h, slot, 0))
            units.append((rb_, h, slot, na))
    return units


def build_program(n_groups=4):
    nc = bacc.Bacc("TRN2", target_bir_lowering=False, debug=False, num_devices=1)

    xT_d = nc.dram_tensor("xT", [D, L], FP32, kind="ExternalInput")
    wq_d = nc.dram_tensor("wqT", [D, 128], FP32, kind="ExternalInput")
    wk_d = nc.dram_tensor("wkT", [D, 128], FP32, kind="ExternalInput")
    wv_d = nc.dram_tensor("wvT", [D, 128], FP32, kind="ExternalInput")
    wo_d = nc.dram_tensor("woT", [128, D], FP32, kind="ExternalInput")
    mneg_d = nc.dram_tensor("mneg", [128, 128], FP32, kind="ExternalInput")
    m01_d = nc.dram_tensor("m01", [128, 128], FP32, kind="ExternalInput")
    ident_d = nc.dram_tensor("ident", [128, 128], FP32, kind="ExternalInput")
    out_d = nc.dram_tensor("out", [L, D], FP32, kind="ExternalOutput")

    with tile.TileContext(nc) as tc:
        with ExitStack() as ctx:
            persist = ctx.enter_context(tc.tile_pool(name="persist", bufs=1))
            qT = persist.tile([128, L], FP32, tag="qT")
            kT = persist.tile([128, L], FP32, tag="kT")
            vt = persist.tile([128, 16, 128], FP16, tag="vt")
            woT = persist.tile([128, D], FP32, tag="woT")
            woT_h = persist.tile([128, D], FP16, tag="woTh")
            mneg = persist.tile([128, 128], FP32, tag="mneg")
            m01 = persist.tile([128, 128], FP32, tag="m01")
            zeros_bf = persist.tile([128, L], BF16, tag="zbf")
            trash_a = persist.tile([128, L], BF16, tag="tra")
            trash_f = persist.tile([128, L], BF16, tag="trf")
            trash_dc = persist.tile([128, L], BF16, tag="trdc")
            # zero-padded transposed-P, one per head: [j, jt, slot, i]
            pth = [persist.tile([128, 16, 4, 128], FP16, tag=f"pth{h}",
                                name=f"pth{h}")
                   for h in range(HPC)]

            NST = 32

            def stat(tag):
                return persist.tile([128, NST], FP32, tag=tag, name=tag)

            sum0, sum1, sum2, sum3 = stat("sum0"), stat("sum1"), stat("sum2"), stat("sum3")
            sumD = stat("sumD")
            sm = stat("sm")
            nh = stat("nh")
            Tt, nT = stat("T"), stat("nT")
            Ft, Ct = stat("F"), stat("C")
            rec, Fm, dlt = stat("rec"), stat("Fm"), stat("dlt")
            tau, ntau = stat("tau"), stat("ntau")
            sump, rz = stat("sump"), stat("rz")
            m8a = persist.tile([128, 8, 8], FP32, tag="m8a")
            m8b = persist.tile([128, 8, 8], FP32, tag="m8b")
            invk = persist.tile([128, 8, 8], FP32, tag="invk")
            t0g = persist.tile([128, 8], FP32, tag="t0g")

            nc.sync.dma_start(mneg[:], mneg_d.ap())
            nc.sync.dma_start(m01[:], m01_d.ap())
            nc.sync.dma_start(woT[:], wo_d.ap())
            nc.gpsimd.tensor_copy(woT_h[:], woT[:])
            nc.vector.memset(zeros_bf[:], 0.0)
            for s in (sum0, sum1, sum2, sum3, sumD):
                nc.vector.memset(s[:], 0.0)
            for kk in range(8):
                nc.vector.memset(invk[:, :, kk], 1.0 / (kk + 1))
            for g in range(n_groups):
                for ui, (rb, h, slot, off) in enumerate(_units_of_group(g)):
                    nc.vector.memset(nh[:, 8 * g + ui:8 * g + ui + 1],
                                     64.0 * (rb + 1))
            for h in range(HPC):
                nc.vector.memset(pth[h][:], 0.0)

            # ---------- phase 1: projections ----------
            with ExitStack() as p1:
                ph1 = p1.enter_context(tc.tile_pool(name="ph1", bufs=1))
                ph1p = p1.enter_context(
                    tc.tile_pool(name="ph1p", bufs=2, space="PSUM"))
                xt = ph1.tile([128, 8, L], FP32, tag="xt")
                xt16 = ph1.tile([128, 8, L], FP16, tag="xt16")
                wqs = ph1.tile([128, 8, 128], FP32, tag="wqs")
                wks = ph1.tile([128, 8, 128], FP32, tag="wks")
                wvs = ph1.tile([128, 8, 128], FP32, tag="wvs")
                wvs16 = ph1.tile([128, 8, 128], FP16, tag="wvs16")
                vTs = ph1.tile([128, 512], FP16, tag="vTs")

                xview = xT_d.ap().rearrange("(c p) n -> p c n", p=128)
                for c in range(8):
                    nc.sync.dma_start(xt[:, c, :], xview[:, c, :])
                nc.sync.dma_start(wqs[:], wq_d.ap().rearrange("(c p) m -> p c m", p=128))
                nc.sync.dma_start(wks[:], wk_d.ap().rearrange("(c p) m -> p c m", p=128))
                nc.sync.dma_start(wvs[:], wv_d.ap().rearrange("(c p) m -> p c m", p=128))

                for c in range(8):
                    if c % 2 == 0:
                        nc.vector.tensor_copy(xt16[:, c, :], xt[:, c, :])
                    else:
                        nc.scalar.copy(xt16[:, c, :], xt[:, c, :])
                nc.gpsimd.tensor_copy(wvs16[:], wvs[:])

                cp_rot = [0]

                def rot_copy(dst, src):
                    if cp_rot[0] % 2 == 0:
                        nc.vector.tensor_copy(dst, src)
                    else:
                        nc.scalar.copy(dst, src)
                    cp_rot[0] += 1

                for dst, wsb in ((qT, wqs), (kT, wks)):
                    for ic in range(4):
                        ps = ph1p.tile([128, 512], FP32, tag="pp")
                        for e in range(8):
                            nc.tensor.matmul(
                                ps[:], wsb[:, e, :],
                                xt[:, e, 512 * ic:512 * (ic + 1)],
                                start=(e == 0), stop=(e == 7))
                        rot_copy(dst[:, 512 * ic:512 * (ic + 1)], ps[:])

                for ic in range(4):
                    ps = ph1p.tile([128, 512], FP32, tag="pp")
                    for e in range(8):
                        nc.tensor.matmul(
                            ps[:], wvs16[:, e, :],
                            xt16[:, e, 512 * ic:512 * (ic + 1)],
                            start=(e == 0), stop=(e == 7))
                    rot_copy(vTs[:], ps[:])
                    nc.sync.dma_start_transpose(
                        vt[:, 4 * ic:4 * ic + 4, :], vTs[:])

            # ---------- phase 2 pools ----------
            s_pool = ctx.enter_context(tc.tile_pool(name="spair", bufs=2))
            p_pool = ctx.enter_context(tc.tile_pool(name="ppair", bufs=2))
            ps_sc = ctx.enter_context(tc.tile_pool(name="ps_sc", bufs=5, space="PSUM"))
            ps_av = ctx.enter_context(tc.tile_pool(name="ps_av", bufs=3, space="PSUM"))
            avh_pool = ctx.enter_context(tc.tile_pool(name="avh", bufs=2))
            avt_pool = ctx.enter_context(tc.tile_pool(name="avt", bufs=4))
            oc_pool = ctx.enter_context(tc.tile_pool(name="oc", bufs=2))
            otb_pool = ctx.enter_context(tc.tile_pool(name="otb", bufs=2))
            wo_pool = ctx.enter_context(tc.tile_pool(name="woout", bufs=2))

            Sg_of = {}

            def emit_phaseA(g):
                units = _units_of_group(g)
                gsl = slice(8 * g, 8 * g + 8)
                Sg = [s_pool.tile([128, PAIR_W], FP32, tag=f"sp{s}",
                                  name=f"sp{s}_{g}") for s in range(4)]
                Sg_of[g] = Sg
                chunk_ctr = [0]
                for ui, (rb, h, slot, off) in enumerate(units):
                    col = 8 * g + ui
                    n = 128 * (rb + 1)
                    full = n - 128
                    S = Sg[slot]
                    qw = qT[64 * h:64 * h + 64, 128 * rb:128 * rb + 128]
                    for ci, c0 in enumerate(range(0, n, 512)):
                        w = min(512, n - c0)
                        ps = ps_sc.tile([128, 512], FP32, tag="sc",
                                        name=f"sc{g}_{ui}_{ci}")
                        nc.tensor.matmul(
                            ps[:, :w], qw, kT[64 * h:64 * h + 64, c0:c0 + w],
                            start=True, stop=True)
                        w_nd = min(w, max(0, full - c0))
                        if w_nd > 0:
                            acc = (sum0, sum1, sum2, sum3)[ci][:, col:col + 1]
                            if chunk_ctr[0] % 3 != 2:
                                nc.scalar.activation(
                                    S[:, off + c0:off + c0 + w_nd], ps[:, :w_nd],
                                    Act.Identity, bias=0.0, accum_out=acc)
                            else:
                                nc.vector.tensor_scalar(
                                    out=S[:, off + c0:off + c0 + w_nd],
                                    in0=ps[:, :w_nd], scalar1=0.0, scalar2=0.0,
                                    op0=Alu.add, op1=Alu.add, accum_out=acc)
                            chunk_ctr[0] += 1
                        if c0 + w > full:
                            ld = full - c0
                            nc.vector.tensor_tensor(
                                S[:, off + full:off + n], ps[:, ld:ld + 128],
                                mneg[:], Alu.add)
                            nc.vector.scalar_tensor_tensor(
                                out=trash_dc[:, :128],
                                in0=ps[:, ld:ld + 128], scalar=1.0, in1=m01[:],
                                op0=Alu.mult, op1=Alu.mult,
                                accum_out=sumD[:, col:col + 1])
                    nc.vector.max(m8a[:, ui, :], S[:, off:off + n])

                # row sums; top8 -> T0
                nc.vector.tensor_tensor(Fm[:, gsl], sum0[:, gsl], sum1[:, gsl], Alu.add)
                nc.vector.tensor_tensor(dlt[:, gsl], sum2[:, gsl], sum3[:, gsl], Alu.add)
                nc.vector.tensor_tensor(sm[:, gsl], Fm[:, gsl], dlt[:, gsl], Alu.add)
                nc.vector.tensor_tensor(sm[:, gsl], sm[:, gsl], sumD[:, gsl], Alu.add)
                nc.vector.tensor_copy(m8b[:], m8a[:])
                nc.vector.tensor_tensor(
                    m8b[:, :, 1:8], m8a[:, :, 1:8], m8a[:, :, 0:7], Alu.add)
                nc.vector.tensor_copy(m8a[:], m8b[:])
                nc.vector.tensor_tensor(
                    m8a[:, :, 2:8], m8b[:, :, 2:8], m8b[:, :, 0:6], Alu.add)
                nc.vector.tensor_copy(m8b[:], m8a[:])
                nc.vector.tensor_tensor(
                    m8b[:, :, 4:8], m8a[:, :, 4:8], m8a[:, :, 0:4], Alu.add)
                nc.vector.tensor_scalar_add(m8b[:], m8b[:], -1.0)
                nc.vector.tensor_tensor(m8b[:], m8b[:], invk[:], Alu.mult)
                nc.vector.tensor_reduce(t0g[:], m8b[:], mybir.AxisListType.X,
                                        Alu.max)
                nc.vector.tensor_copy(Tt[:, gsl], t0g[:])
                nc.vector.tensor_scalar_mul(nT[:, gsl], t0g[:], -1.0)

            def emit_F(g, ui, rb, h, slot, off):
                col = 8 * g + ui
                n = 128 * (rb + 1)
                S = Sg_of[g][slot]
                if ui < 5:
                    nc.scalar.activation(
                        trash_a[:, :n], S[:, off:off + n], Act.Relu,
                        bias=nT[:, col:col + 1],
                        accum_out=Ft[:, col:col + 1])
                else:
                    nc.vector.scalar_tensor_tensor(
                        out=trash_f[:, :n], in0=S[:, off:off + n],
                        scalar=nT[:, col:col + 1], in1=zeros_bf[:, :n],
                        op0=Alu.add, op1=Alu.max,
                        accum_out=Ft[:, col:col + 1])

            def emit_cnt(g, ui, rb, h, slot, off):
                col = 8 * g + ui
                n = 128 * (rb + 1)
                S = Sg_of[g][slot]
                if ui < 5:
                    nc.vector.tensor_scalar(
                        out=trash_dc[:, :n], in0=S[:, off:off + n],
                        scalar1=Tt[:, col:col + 1], scalar2=0.0,
                        op0=Alu.is_gt, op1=Alu.add,
                        accum_out=Ct[:, col:col + 1])
                else:
                    # Sign trick: masked (-1e30) entries count -1, so
                    # cnt = 0.5*A + n/2 with n the full padded width.
                    nc.scalar.activation(
                        trash_a[:, :n], S[:, off:off + n], Act.Sign,
                        bias=nT[:, col:col + 1],
                        accum_out=Ct[:, col:col + 1])

            def cnt_fixup(g):
                hsl = slice(8 * g + 5, 8 * g + 8)
                nc.vector.scalar_tensor_tensor(
                    out=Ct[:, hsl], in0=Ct[:, hsl], scalar=0.5, in1=nh[:, hsl],
                    op0=Alu.mult, op1=Alu.add)

            def emit_newton_iter(g, it):
                units = _units_of_group(g)
                gsl = slice(8 * g, 8 * g + 8)
                for ui, (rb, h, slot, off) in enumerate(units):
                    emit_F(g, ui, rb, h, slot, off)
                    emit_cnt(g, ui, rb, h, slot, off)
                cnt_fixup(g)
                nc.vector.tensor_scalar_max(Ct[:, gsl], Ct[:, gsl], 1.0)
                nc.vector.reciprocal(rec[:, gsl], Ct[:, gsl])
                nc.vector.tensor_scalar_add(Fm[:, gsl], Ft[:, gsl], -1.0)
                nc.vector.tensor_tensor(dlt[:, gsl], Fm[:, gsl], rec[:, gsl], Alu.mult)
                nc.vector.tensor_tensor(Tt[:, gsl], Tt[:, gsl], dlt[:, gsl], Alu.add)
                nc.vector.tensor_tensor(nT[:, gsl], nT[:, gsl], dlt[:, gsl], Alu.subtract)

            def emit_final(g):
                units = _units_of_group(g)
                gsl = slice(8 * g, 8 * g + 8)
                for ui, (rb, h, slot, off) in enumerate(units):
                    emit_cnt(g, ui, rb, h, slot, off)
                cnt_fixup(g)
                nc.vector.tensor_scalar_max(Ct[:, gsl], Ct[:, gsl], 1.0)
                nc.vector.reciprocal(rec[:, gsl], Ct[:, gsl])
                nc.vector.tensor_scalar_add(Fm[:, gsl], sm[:, gsl], -1.0)
                nc.vector.tensor_tensor(tau[:, gsl], Fm[:, gsl], rec[:, gsl], Alu.mult)
                nc.vector.tensor_scalar_mul(ntau[:, gsl], tau[:, gsl], -1.0)

            def emit_P(g):
                units = _units_of_group(g)
                Pg = [p_pool.tile([128, PAIR_W], FP16, tag=f"pp{s}",
                                  name=f"pp{s}_{g}") for s in range(4)]
                for ui, (rb, h, slot, off) in enumerate(units):
                    col = 8 * g + ui
                    n = 128 * (rb + 1)
                    S, P = Sg_of[g][slot], Pg[slot]
                    if ui < 6:
                        nc.scalar.activation(
                            P[:, off:off + n], S[:, off:off + n], Act.Relu,
                            bias=ntau[:, col:col + 1],
                            accum_out=sump[:, col:col + 1])
                    else:
                        nc.vector.tensor_scalar(
                            out=P[:, off:off + n], in0=S[:, off:off + n],
                            scalar1=ntau[:, col:col + 1], scalar2=0.0,
                            op0=Alu.add, op1=Alu.max)
                        nc.vector.tensor_scalar(
                            out=trash_f[:, :n], in0=P[:, off:off + n],
                            scalar1=0.0, scalar2=0.0,
                            op0=Alu.add, op1=Alu.add,
                            accum_out=sump[:, col:col + 1])
                    nc.vector.tensor_scalar_add(
                        Fm[:, col:col + 1], sump[:, col:col + 1], 1.0e-10)
                    nc.vector.reciprocal(rz[:, col:col + 1], Fm[:, col:col + 1])
                return Pg

            def emit_tail(g, Pg):
                units = _units_of_group(g)
                # slot -> nt for this group; stale-block zeroing for shrinking slots
                nts = {}
                for ui, (rb, h, slot, off) in enumerate(units):
                    if h == 0:
                        nts[slot // 2 * 2 + (1 if off > 0 else 0)] = rb + 1
                # slots in pth layout: index by (pair_local, a/b) = 0..3
                # unit slot s in Sg corresponds to pth slot: derive from units
                # pth slot assignment: use (pi_local*2 + is_b)
                pth_slot = {}
                for ui, (rb, h, slot, off) in enumerate(units):
                    pi_local = slot // 2
                    is_b = 1 if off > 0 else 0
                    pth_slot[ui] = pi_local * 2 + is_b

                if g > 0:
                    prev = _units_of_group(g - 1)
                    for ui, (rb, h, slot, off) in enumerate(units):
                        psl = pth_slot[ui]
                        (prb, _, _, poff) = prev[ui]
                        nt, pnt = rb + 1, prb + 1
                        if nt < pnt:
                            # zero the now-stale jt blocks
                            nc.vector.memset(
                                pth[h][:, nt:pnt, psl, :], 0.0)

                # P^T via DMA xbar into pth
                for ui, (rb, h, slot, off) in enumerate(units):
                    nt = rb + 1
                    psl = pth_slot[ui]
                    nc.sync.dma_start_transpose(
                        pth[h][:, 0:nt, psl, :],
                        Pg[slot][:, off:off + 128 * nt])

                # AV: one matmul per (head, jt), 512-wide moving
                maxnt = max(rb + 1 for (rb, _, _, _) in units)
                avps = {}
                for h in range(HPC):
                    avps[h] = ps_av.tile([128, 512], FP32, tag="av",
                                         name=f"av{g}_{h}")
                    for jt in range(maxnt):
                        nc.tensor.matmul(
                            avps[h][:64, :],
                            vt[:, jt, 64 * h:64 * h + 64],
                            pth[h][:, jt, :, :],
                            start=(jt == 0), stop=(jt == maxnt - 1))

                # avps [64d, 4slot*128i] -> fp16 -> per-block DMA transpose
                outc_of_rb = {}
                avh = {}
                for h in range(HPC):
                    avh[h] = avh_pool.tile([128, 512], FP16, tag="avh",
                                           name=f"avh{g}_{h}")
                    if h == 0:
                        nc.scalar.copy(avh[h][:64, :], avps[h][:64, :])
                    else:
                        nc.vector.tensor_copy(avh[h][:64, :], avps[h][:64, :])

                for ui, (rb, h, slot, off) in enumerate(units):
                    col = 8 * g + ui
                    psl = pth_slot[ui]
                    avt = avt_pool.tile([128, 64], FP16, tag="avt",
                                        name=f"avt{g}_{ui}")
                    nc.sync.dma_start_transpose(
                        avt[:], avh[h][:64, 128 * psl:128 * (psl + 1)])
                    if rb not in outc_of_rb:
                        outc_of_rb[rb] = oc_pool.tile(
                            [128, 128], FP16, tag=f"oc{ui % 2}",
                            name=f"oc{g}_{rb}")
                    outc = outc_of_rb[rb]
                    nc.scalar.activation(
                        outc[:, 64 * h:64 * h + 64], avt[:], Act.Copy,
                        bias=0.0, scale=rz[:, col:col + 1])

                    if h == 1:
                        wo_out = wo_pool.tile([128, D], FP32, tag="wod",
                                              name=f"wod{g}_{rb}")
                        otb = otb_pool.tile([128, 128], FP16, tag="otbh",
                                            name=f"otb{g}_{rb}")
                        nc.sync.dma_start_transpose(otb[:], outc[:])
                        for oc2 in range(2):
                            wps = ps_av.tile([128, 512], FP32, tag="av",
                                             name=f"wo{g}_{rb}_{oc2}")
                            nc.tensor.matmul(
                                wps[:], otb[:],
                                woT_h[:, 512 * oc2:512 * (oc2 + 1)],
                                start=True, stop=True)
                            if oc2 == 0:
                                nc.scalar.copy(wo_out[:, :512], wps[:])
                            else:
                                nc.vector.tensor_copy(wo_out[:, 512:], wps[:])
                        nc.sync.dma_start(
                            out_d.ap()[128 * rb:128 * (rb + 1), :], wo_out[:])

            # ---------- pipelined emission ----------
            for ui in range(8):
                emit_phaseA_unit(0, ui)
            emit_phaseA_finish(0)
            for g in range(n_groups):
                units = _units_of_group(g)
                gsl = slice(8 * g, 8 * g + 8)
                for it in range(N_ITERS):
                    emit_newton_iter(g, it)
                    # interleave next group's phase A between iterations so
                    # the PE has score matmuls to chew on during Newton
                    if g + 1 < n_groups:
                        for ui in range(3 * it, min(3 * it + 3, 8)):
                            emit_phaseA_unit(g + 1, ui)
                emit_final(g)
                Pg = emit_P(g)
                if g + 1 < n_groups:
                    for ui in range(3 * N_ITERS, 8):
                        emit_phaseA_unit(g + 1, ui)
                    emit_phaseA_finish(g + 1)
                emit_tail(g, Pg)

    nc.compile()
    return nc


_CACHE = {}


def _get_nc():
    if "nc" not in _CACHE:
        _CACHE["nc"] = build_program()
    return _CACHE["nc"]


def _host_inputs(x, Wq, Wk, Wv, Wo):
    xT = np.ascontiguousarray(x[0].T).astype(np.float32)
    ii = np.arange(128)
    mneg = np.where(ii[None, :] > ii[:, None], np.float32(NEG_BIG),
                    np.float32(0.0)).astype(np.float32)
    m01 = (ii[None, :] <= ii[:, None]).astype(np.float32)
    in_maps = []
    for c in range(N_CORES):
        hsl = slice(128 * c, 128 * (c + 1))
        in_maps.append({
            "xT": xT,
            "wqT": np.ascontiguousarray((Wq[hsl] * np.float32(SCALE)).T).astype(np.float32),
            "wkT": np.ascontiguousarray(Wk[hsl].T).astype(np.float32),
            "wvT": np.ascontiguousarray(Wv[hsl].T).astype(np.float32),
            "woT": np.ascontiguousarray(Wo[:, hsl].T).astype(np.float32),
            "mneg": mneg,
            "m01": m01,
            "ident": np.eye(128, dtype=np.float32),
        })
    return in_maps


def kernel(x, Wq, Wk, Wv, Wo, _trace=False):
    nc = _get_nc()
    in_maps = _host_inputs(np.asarray(x), np.asarray(Wq), np.asarray(Wk),
                           np.asarray(Wv), np.asarray(Wo))
    res = run_bass_kernel_spmd(nc, in_maps, core_ids=list(range(N_CORES)),
                               trace=_trace)
    out = np.zeros((L, D), np.float32)
    for c in range(N_CORES):
        out += res.results[c]["out"]
    if _trace:
        _CACHE["last_results"] = res
    return out.reshape(1, L, D)


# revision 8
# speedup vs baseline: 2.1514x; 1.0492x over previous
"""Entmax attention Trainium2 kernel v5 (8-core SPMD, head-parallel).

HW-calibrated design (v4 trace: DVE 86%, PE 76% w/ 365us of gaps, ACT 43%):
- top-8 seed T0 = max_k (cumsum_k-1)/k, 3 plain Newton iters (100% exact
  k_support at 3 in numpy), final fp32 count.
- Per-iteration engine split calibrated to HW (everything ~1 elem/cycle):
  F: units 0-4 ACT Relu+accum, 5-7 DVE stt; cnt: units 0-4 DVE is_gt+accum,
  5-7 ACT Sign+accum with (A + n)/2 fixup (masked entries count as -1).
- Pipelined emission: phaseA(g+1) is emitted before the AV/Wo tail of g so
  the PE's in-order queue can run scores g+1 and AV g during Newton phases.
- AV batched jt-major: P^T DMA-xbar-transposed into a zero-padded
  [128, 16jt, 4slot, 128] layout per head; one matmul per (head, jt) with a
  512-wide moving operand accumulating all 4 row-blocks at once
  (68 matmuls+ldweights per group -> ~29).
"""
import numpy as np
from contextlib import ExitStack

import concourse.bass as bass
import concourse.tile as tile
import concourse.mybir as mybir
from concourse import bacc
from concourse.bass_utils import run_bass_kernel_spmd

L = 2048
D = 1024
H = 16
HD = 64
N_CORES = 8
HPC = 2
SCALE = float(HD) ** -0.5

FP32 = mybir.dt.float32
FP16 = mybir.dt.float16
BF16 = mybir.dt.bfloat16
Alu = mybir.AluOpType
Act = mybir.ActivationFunctionType

N_ITERS = 3
NEG_BIG = -1.0e30
MAX_INIT = -3.0e38

RB_PAIRS = [(0, 15), (4, 11), (1, 14), (5, 10), (2, 13), (6, 9), (3, 12), (7, 8)]
PAIR_W = 17 * 128  # 2176


def _units_of_group(g):
    units = []
    p0, p1 = RB_PAIRS[2 * g], RB_PAIRS[2 * g + 1]
    for h in range(HPC):
        for pi_local, (ra, rb_) in enumerate((p0, p1)):
            slot = 2 * pi_local + h
            na = 128 * (ra + 1)
            units.append((ra, h, slot, 0))
            units.append((rb_, h, slot, na))
    return units


def build_program(n_groups=4):
    nc = bacc.Bacc("TRN2", target_bir_lowering=False, debug=False, num_devices=1)

    xT_d = nc.dram_tensor("xT", [D, L], FP32, kind="ExternalInput")
    wq_d = nc.dram_tensor("wqT", [D, 128], FP32, kind="ExternalInput")
    wk_d = nc.dram_tensor("wkT", [D, 128], FP32, kind="ExternalInput")
    wv_d = nc.dram_tensor("wvT", [D, 128], FP32, kind="ExternalInput")
    wo_d = nc.dram_tensor("woT", [128, D], FP32, kind="ExternalInput")
    mneg_d = nc.dram_tensor("mneg", [128, 128], FP32, kind="ExternalInput")
    m01_d = nc.dram_tensor("m01", [128, 128], FP32, kind="ExternalInput")
    ident_d = nc.dram_tensor("ident", [128, 128], FP32, kind="ExternalInput")
    mrow_d = nc.dram_tensor("mrow", [128, 1], FP32, kind="ExternalInput")
    out_d = nc.dram_tensor("out", [L, D], FP32, kind="ExternalOutput")

    with tile.TileContext(nc) as tc:
        with ExitStack() as ctx:
            persist = ctx.enter_context(tc.tile_pool(name="persist", bufs=1))
            qT = persist.tile([128, L], FP32, tag="qT")
            kT = persist.tile([128, L], FP32, tag="kT")
            vt = persist.tile([128, 16, 128], FP16, tag="vt")
            woT_h = persist.tile([128, D], FP16, tag="woTh")
            mneg = persist.tile([128, 128], FP32, tag="mneg")
            m01 = persist.tile([128, 128], FP32, tag="m01")
            zeros_bf = persist.tile([128, L], BF16, tag="zbf")
            trash_a = persist.tile([128, L], BF16, tag="tra")
            trash_dc = persist.tile([128, L], BF16, tag="trdc")
            trash_f = trash_dc
            # zero-padded transposed-P, one per head: [j, jt, slot, i]
            pth = [persist.tile([128, 16, 4, 128], FP16, tag=f"pth{h}",
                                name=f"pth{h}")
                   for h in range(HPC)]

            NST = 32

            def stat(tag):
                return persist.tile([128, NST], FP32, tag=tag, name=tag)

            sum0, sum1, sum2, sum3 = stat("sum0"), stat("sum1"), stat("sum2"), stat("sum3")
            sumD = stat("sumD")
            sm = stat("sm")
            nh = stat("nh")
            Tt, nT = stat("T"), stat("nT")
            Ft, Ct = stat("F"), stat("C")
            rec, Fm, dlt = stat("rec"), stat("Fm"), stat("dlt")
            tau, ntau = stat("tau"), stat("ntau")
            sump, rz = stat("sump"), stat("rz")
            m8a = persist.tile([128, 8, 8], FP32, tag="m8a")
            m8b = persist.tile([128, 8, 8], FP32, tag="m8b")
            invk = persist.tile([128, 8, 8], FP32, tag="invk")
            t0g = persist.tile([128, 8], FP32, tag="t0g")

            mrow = persist.tile([128, 1], FP32, tag="mrow")
            nc.sync.dma_start(mrow[:], mrow_d.ap())
            nc.sync.dma_start(mneg[:], mneg_d.ap())
            nc.sync.dma_start(m01[:], m01_d.ap())

            nc.vector.memset(zeros_bf[:], 0.0)
            for s in (sum0, sum1, sum2, sum3, sumD):
                nc.vector.memset(s[:], 0.0)
            for kk in range(8):
                nc.vector.memset(invk[:, :, kk], 1.0 / (kk + 1))
            for g in range(n_groups):
                for ui, (rb, h, slot, off) in enumerate(_units_of_group(g)):
                    nc.vector.memset(nh[:, 8 * g + ui:8 * g + ui + 1],
                                     64.0 * (rb + 1))
            for h in range(HPC):
                nc.vector.memset(pth[h][:], 0.0)

            # ---------- phase 1: projections ----------
            _vproj_holder = []
            with ExitStack() as p1:
                ph1 = p1.enter_context(tc.tile_pool(name="ph1", bufs=1))
                ph1p = p1.enter_context(
                    tc.tile_pool(name="ph1p", bufs=2, space="PSUM"))
                xt = ph1.tile([128, 8, L], FP32, tag="xt")
                xt16 = ph1.tile([128, 8, L], FP16, tag="xt16")
                wqs = ph1.tile([128, 8, 128], FP32, tag="wqs")
                wks = ph1.tile([128, 8, 128], FP32, tag="wks")
                wvs = ph1.tile([128, 8, 128], FP32, tag="wvs")
                wvs16 = ph1.tile([128, 8, 128], FP16, tag="wvs16")
                vTs = ph1.tile([128, 512], FP16, tag="vTs")
                woT32 = ph1.tile([128, D], FP32, tag="woT32")
                nc.sync.dma_start(woT32[:], wo_d.ap())
                nc.gpsimd.tensor_copy(woT_h[:], woT32[:])

                nc.sync.dma_start(wqs[:], wq_d.ap().rearrange("(c p) m -> p c m", p=128))
                nc.sync.dma_start(wks[:], wk_d.ap().rearrange("(c p) m -> p c m", p=128))
                nc.sync.dma_start(wvs[:], wv_d.ap().rearrange("(c p) m -> p c m", p=128))
                xview = xT_d.ap().rearrange("(c p) n -> p c n", p=128)
                for c in range(8):
                    nc.sync.dma_start(xt[:, c, :], xview[:, c, :])

                for c in range(8):
                    if c % 2 == 0:
                        nc.vector.tensor_copy(xt16[:, c, :], xt[:, c, :])
                    else:
                        nc.scalar.copy(xt16[:, c, :], xt[:, c, :])
                nc.gpsimd.tensor_copy(wvs16[:], wvs[:])

                cp_rot = [0]

                def rot_copy(dst, src):
                    if cp_rot[0] % 2 == 0:
                        nc.vector.tensor_copy(dst, src)
                    else:
                        nc.scalar.copy(dst, src)
                    cp_rot[0] += 1

                for dst, wsb in ((qT, wqs), (kT, wks)):
                    for ic in range(4):
                        ps = ph1p.tile([128, 512], FP32, tag="pp")
                        for e in range(8):
                            nc.tensor.matmul(
                                ps[:], wsb[:, e, :],
                                xt[:, e, 512 * ic:512 * (ic + 1)],
                                start=(e == 0), stop=(e == 7))
                        rot_copy(dst[:, 512 * ic:512 * (ic + 1)], ps[:])

                def emit_vproj():
                    for ic in range(4):
                        ps = ph1p.tile([128, 512], FP32, tag="pp")
                        for e in range(8):
                            nc.tensor.matmul(
                                ps[:], wvs16[:, e, :],
                                xt16[:, e, 512 * ic:512 * (ic + 1)],
                                start=(e == 0), stop=(e == 7))
                        rot_copy(vTs[:], ps[:])
                        nc.sync.dma_start_transpose(
                            vt[:, 4 * ic:4 * ic + 4, :], vTs[:])
                _vproj_holder.append(emit_vproj)

            # ---------- phase 2 pools ----------
            s_pool = ctx.enter_context(tc.tile_pool(name="spair", bufs=3))
            p_pool = ctx.enter_context(tc.tile_pool(name="ppair", bufs=1))
            ps_sc = ctx.enter_context(tc.tile_pool(name="ps_sc", bufs=5, space="PSUM"))
            ps_av = ctx.enter_context(tc.tile_pool(name="ps_av", bufs=3, space="PSUM"))
            avh_pool = ctx.enter_context(tc.tile_pool(name="avh", bufs=2))
            avt_pool = ctx.enter_context(tc.tile_pool(name="avt", bufs=4))
            oc_pool = ctx.enter_context(tc.tile_pool(name="oc", bufs=2))
            otb_pool = ctx.enter_context(tc.tile_pool(name="otb", bufs=2))
            wo_pool = ctx.enter_context(tc.tile_pool(name="woout", bufs=2))

            Sg_of = {}

            def emit_phaseA(g):
                units = _units_of_group(g)
                gsl = slice(8 * g, 8 * g + 8)
                Sg = [s_pool.tile([128, PAIR_W], FP32, tag=f"sp{s}",
                                  name=f"sp{s}_{g}") for s in range(4)]
                Sg_of[g] = Sg
                chunk_ctr = [0]
                for ui, (rb, h, slot, off) in enumerate(units):
                    col = 8 * g + ui
                    n = 128 * (rb + 1)
                    full = n - 128
                    S = Sg[slot]
                    qw = qT[64 * h:64 * h + 64, 128 * rb:128 * rb + 128]
                    for ci, c0 in enumerate(range(0, n, 512)):
                        w = min(512, n - c0)
                        ps = ps_sc.tile([128, 512], FP32, tag="sc",
                                        name=f"sc{g}_{ui}_{ci}")
                        nc.tensor.matmul(
                            ps[:, :w], qw, kT[64 * h:64 * h + 64, c0:c0 + w],
                            start=True, stop=True)
                        w_nd = min(w, max(0, full - c0))
                        if w_nd > 0:
                            acc = (sum0, sum1, sum2, sum3)[ci][:, col:col + 1]
                            if chunk_ctr[0] % 3 != 2:
                                nc.scalar.activation(
                                    S[:, off + c0:off + c0 + w_nd], ps[:, :w_nd],
                                    Act.Identity, bias=0.0, accum_out=acc)
                            else:
                                nc.vector.tensor_scalar(
                                    out=S[:, off + c0:off + c0 + w_nd],
                                    in0=ps[:, :w_nd], scalar1=0.0, scalar2=0.0,
                                    op0=Alu.add, op1=Alu.add, accum_out=acc)
                            chunk_ctr[0] += 1
                        if c0 + w > full:
                            ld = full - c0
                            nc.vector.tensor_tensor(
                                S[:, off + full:off + n], ps[:, ld:ld + 128],
                                mneg[:], Alu.add)
                            nc.vector.scalar_tensor_tensor(
                                out=trash_dc[:, :128],
                                in0=ps[:, ld:ld + 128], scalar=1.0, in1=m01[:],
                                op0=Alu.mult, op1=Alu.mult,
                                accum_out=sumD[:, col:col + 1])
                    nc.vector.max(m8a[:, ui, :], S[:, off:off + n])

                # row sums; top8 -> T0
                nc.vector.tensor_tensor(Fm[:, gsl], sum0[:, gsl], sum1[:, gsl], Alu.add)
                nc.vector.tensor_tensor(dlt[:, gsl], sum2[:, gsl], sum3[:, gsl], Alu.add)
                nc.vector.tensor_tensor(sm[:, gsl], Fm[:, gsl], dlt[:, gsl], Alu.add)
                nc.vector.tensor_tensor(sm[:, gsl], sm[:, gsl], sumD[:, gsl], Alu.add)
                nc.vector.tensor_copy(m8b[:], m8a[:])
                nc.vector.tensor_tensor(
                    m8b[:, :, 1:8], m8a[:, :, 1:8], m8a[:, :, 0:7], Alu.add)
                nc.vector.tensor_copy(m8a[:], m8b[:])
                nc.vector.tensor_tensor(
                    m8a[:, :, 2:8], m8b[:, :, 2:8], m8b[:, :, 0:6], Alu.add)
                nc.vector.tensor_copy(m8b[:], m8a[:])
                nc.vector.tensor_tensor(
                    m8b[:, :, 4:8], m8a[:, :, 4:8], m8a[:, :, 0:4], Alu.add)
                nc.vector.tensor_scalar_add(m8b[:], m8b[:], -1.0)
                nc.vector.tensor_tensor(m8b[:], m8b[:], invk[:], Alu.mult)
                nc.vector.tensor_reduce(t0g[:], m8b[:], mybir.AxisListType.X,
                                        Alu.max)
                nc.vector.tensor_copy(Tt[:, gsl], t0g[:])
                nc.vector.tensor_scalar_mul(nT[:, gsl], t0g[:], -1.0)

            def emit_F(g, ui, rb, h, slot, off):
                col = 8 * g + ui
                n = 128 * (rb + 1)
                S = Sg_of[g][slot]
                if ui < 5:
                    nc.scalar.activation(
                        trash_a[:, :n], S[:, off:off + n], Act.Relu,
                        bias=nT[:, col:col + 1],
                        accum_out=Ft[:, col:col + 1])
                else:
                    nc.vector.scalar_tensor_tensor(
                        out=trash_f[:, :n], in0=S[:, off:off + n],
                        scalar=nT[:, col:col + 1], in1=zeros_bf[:, :n],
                        op0=Alu.add, op1=Alu.max,
                        accum_out=Ft[:, col:col + 1])

            def emit_cnt(g, ui, rb, h, slot, off):
                col = 8 * g + ui
                n = 128 * (rb + 1)
                S = Sg_of[g][slot]
                if ui < 5:
                    nc.vector.tensor_scalar(
                        out=trash_dc[:, :n], in0=S[:, off:off + n],
                        scalar1=Tt[:, col:col + 1], scalar2=0.0,
                        op0=Alu.is_gt, op1=Alu.add,
                        accum_out=Ct[:, col:col + 1])
                else:
                    # Sign trick: masked (-1e30) entries count -1, so
                    # cnt = 0.5*A + n/2 with n the full padded width.
                    nc.scalar.activation(
                        trash_a[:, :n], S[:, off:off + n], Act.Sign,
                        bias=nT[:, col:col + 1],
                        accum_out=Ct[:, col:col + 1])

            def cnt_fixup(g):
                hsl = slice(8 * g + 5, 8 * g + 8)
                nc.vector.scalar_tensor_tensor(
                    out=Ct[:, hsl], in0=Ct[:, hsl], scalar=0.5, in1=nh[:, hsl],
                    op0=Alu.mult, op1=Alu.add)

            def emit_newton_iter(g, it):
                units = _units_of_group(g)
                gsl = slice(8 * g, 8 * g + 8)
                for ui, (rb, h, slot, off) in enumerate(units):
                    emit_F(g, ui, rb, h, slot, off)
                    emit_cnt(g, ui, rb, h, slot, off)
                cnt_fixup(g)
                nc.vector.tensor_scalar_max(Ct[:, gsl], Ct[:, gsl], 1.0)
                nc.vector.reciprocal(rec[:, gsl], Ct[:, gsl])
                nc.vector.tensor_scalar_add(Fm[:, gsl], Ft[:, gsl], -1.0)
                nc.vector.tensor_tensor(dlt[:, gsl], Fm[:, gsl], rec[:, gsl], Alu.mult)
                nc.vector.tensor_tensor(Tt[:, gsl], Tt[:, gsl], dlt[:, gsl], Alu.add)
                nc.vector.tensor_tensor(nT[:, gsl], nT[:, gsl], dlt[:, gsl], Alu.subtract)

            def emit_final(g):
                units = _units_of_group(g)
                gsl = slice(8 * g, 8 * g + 8)
                for ui, (rb, h, slot, off) in enumerate(units):
                    emit_cnt(g, ui, rb, h, slot, off)
                cnt_fixup(g)
                nc.vector.tensor_scalar_max(Ct[:, gsl], Ct[:, gsl], 1.0)
                nc.vector.reciprocal(rec[:, gsl], Ct[:, gsl])
                nc.vector.tensor_scalar_add(Fm[:, gsl], sm[:, gsl], -1.0)
                nc.vector.tensor_tensor(tau[:, gsl], Fm[:, gsl], rec[:, gsl], Alu.mult)
                nc.vector.tensor_scalar_mul(ntau[:, gsl], tau[:, gsl], -1.0)

            def emit_P(g):
                units = _units_of_group(g)
                Pg = [p_pool.tile([128, PAIR_W], FP16, tag=f"pp{s}",
                                  name=f"pp{s}_{g}") for s in range(4)]
                for ui, (rb, h, slot, off) in enumerate(units):
                    col = 8 * g + ui
                    n = 128 * (rb + 1)
                    S, P = Sg_of[g][slot], Pg[slot]
                    if ui < 6:
                        nc.scalar.activation(
                            P[:, off:off + n], S[:, off:off + n], Act.Relu,
                            bias=ntau[:, col:col + 1],
                            accum_out=sump[:, col:col + 1])
                    else:
                        nc.vector.tensor_scalar(
                            out=P[:, off:off + n], in0=S[:, off:off + n],
                            scalar1=ntau[:, col:col + 1], scalar2=0.0,
                            op0=Alu.add, op1=Alu.max)
                        nc.vector.tensor_scalar(
                            out=trash_f[:, :n], in0=P[:, off:off + n],
                            scalar1=0.0, scalar2=0.0,
                            op0=Alu.add, op1=Alu.add,
                            accum_out=sump[:, col:col + 1])
                    nc.vector.tensor_scalar_add(
                        Fm[:, col:col + 1], sump[:, col:col + 1], 1.0e-10)
                    nc.vector.reciprocal(rz[:, col:col + 1], Fm[:, col:col + 1])
                return Pg

            def emit_tail(g, Pg):
                units = _units_of_group(g)
                # slot -> nt for this group; stale-block zeroing for shrinking slots
                nts = {}
                for ui, (rb, h, slot, off) in enumerate(units):
                    if h == 0:
                        nts[slot // 2 * 2 + (1 if off > 0 else 0)] = rb + 1
                # slots in pth layout: index by (pair_local, a/b) = 0..3
                # unit slot s in Sg corresponds to pth slot: derive from units
                # pth slot assignment: use (pi_local*2 + is_b)
                pth_slot = {}
                for ui, (rb, h, slot, off) in enumerate(units):
                    pi_local = slot // 2
                    is_b = 1 if off > 0 else 0
                    pth_slot[ui] = pi_local * 2 + is_b

                if g > 0:
                    prev = _units_of_group(g - 1)
                    for ui, (rb, h, slot, off) in enumerate(units):
                        psl = pth_slot[ui]
                        (prb, _, _, poff) = prev[ui]
                        nt, pnt = rb + 1, prb + 1
                        if nt < pnt:
                            # zero the now-stale jt blocks
                            nc.vector.memset(
                                pth[h][:, nt:pnt, psl, :], 0.0)

                # P^T via DMA xbar into pth
                for ui, (rb, h, slot, off) in enumerate(units):
                    nt = rb + 1
                    psl = pth_slot[ui]
                    nc.sync.dma_start_transpose(
                        pth[h][:, 0:nt, psl, :],
                        Pg[slot][:, off:off + 128 * nt])

                # AV: one matmul per (head, jt), 512-wide moving
                maxnt = max(rb + 1 for (rb, _, _, _) in units)
                avps = {}
                for h in range(HPC):
                    avps[h] = ps_av.tile([128, 512], FP32, tag="av",
                                         name=f"av{g}_{h}")
                    for jt in range(maxnt):
                        nc.tensor.matmul(
                            avps[h][:64, :],
                            vt[:, jt, 64 * h:64 * h + 64],
                            pth[h][:, jt, :, :],
                            start=(jt == 0), stop=(jt == maxnt - 1))

                # avps [64d, 4slot*128i] -> fp16 -> per-block DMA transpose
                outc_of_rb = {}
                avh = {}
                for h in range(HPC):
                    avh[h] = avh_pool.tile([128, 512], FP16, tag="avh",
                                           name=f"avh{g}_{h}")
                    if h == 0:
                        nc.scalar.copy(avh[h][:64, :], avps[h][:64, :])
                    else:
                        nc.vector.tensor_copy(avh[h][:64, :], avps[h][:64, :])

                for ui, (rb, h, slot, off) in enumerate(units):
                    col = 8 * g + ui
                    psl = pth_slot[ui]
                    avt = avt_pool.tile([128, 64], FP16, tag="avt",
                                        name=f"avt{g}_{ui}")
                    nc.sync.dma_start_transpose(
                        avt[:], avh[h][:64, 128 * psl:128 * (psl + 1)])
                    if rb not in outc_of_rb:
                        outc_of_rb[rb] = oc_pool.tile(
                            [128, 128], FP16, tag=f"oc{ui % 2}",
                            name=f"oc{g}_{rb}")
                    outc = outc_of_rb[rb]
                    nc.scalar.activation(
                        outc[:, 64 * h:64 * h + 64], avt[:], Act.Copy,
                        bias=0.0, scale=rz[:, col:col + 1])

                    if h == 1:
                        wo_out = wo_pool.tile([128, D], FP32, tag="wod",
                                              name=f"wod{g}_{rb}")
                        otb = otb_pool.tile([128, 128], FP16, tag="otbh",
                                            name=f"otb{g}_{rb}")
                        nc.sync.dma_start_transpose(otb[:], outc[:])
                        for oc2 in range(2):
                            wps = ps_av.tile([128, 512], FP32, tag="av",
                                             name=f"wo{g}_{rb}_{oc2}")
                            nc.tensor.matmul(
                                wps[:], otb[:],
                                woT_h[:, 512 * oc2:512 * (oc2 + 1)],
                                start=True, stop=True)
                            if oc2 == 0:
                                nc.scalar.copy(wo_out[:, :512], wps[:])
                            else:
                                nc.vector.tensor_copy(wo_out[:, 512:], wps[:])
                        nc.sync.dma_start(
                            out_d.ap()[128 * rb:128 * (rb + 1), :], wo_out[:])

            # ---------- pipelined emission ----------
            for ui in range(8):
                emit_phaseA_unit(0, ui)
            emit_phaseA_finish(0)
            for g in range(n_groups):
                units = _units_of_group(g)
                gsl = slice(8 * g, 8 * g + 8)
                for it in range(N_ITERS):
                    emit_newton_iter(g, it)
                    # interleave next group's phase A between iterations so
                    # the PE has score matmuls to chew on during Newton
                    if g + 1 < n_groups:
                        for ui in range(3 * it, min(3 * it + 3, 8)):
                            emit_phaseA_unit(g + 1, ui)
                emit_final(g)
                Pg = emit_P(g)
                if g + 1 < n_groups:
                    for ui in range(3 * N_ITERS, 8):
                        emit_phaseA_unit(g + 1, ui)
                    emit_phaseA_finish(g + 1)
                emit_tail(g, Pg)

    nc.compile()
    return nc


_CACHE = {}


def _get_nc():
    if "nc" not in _CACHE:
        _CACHE["nc"] = build_program()
    return _CACHE["nc"]


def _host_inputs(x, Wq, Wk, Wv, Wo):
    xT = np.ascontiguousarray(x[0].T).astype(np.float32)
    ii = np.arange(128)
    mneg = np.where(ii[None, :] > ii[:, None], np.float32(NEG_BIG),
                    np.float32(0.0)).astype(np.float32)
    m01 = (ii[None, :] <= ii[:, None]).astype(np.float32)
    in_maps = []
    for c in range(N_CORES):
        hsl = slice(128 * c, 128 * (c + 1))
        in_maps.append({
            "xT": xT,
            "wqT": np.ascontiguousarray((Wq[hsl] * np.float32(SCALE)).T).astype(np.float32),
            "wkT": np.ascontiguousarray(Wk[hsl].T).astype(np.float32),
            "wvT": np.ascontiguousarray(Wv[hsl].T).astype(np.float32),
            "woT": np.ascontiguousarray(Wo[:, hsl].T).astype(np.float32),
            "mneg": mneg,
            "m01": m01,
            "ident": np.eye(128, dtype=np.float32),
            "mrow": (60000.0 * (127 - np.arange(128, dtype=np.float32))
                     ).reshape(128, 1).astype(np.float32),
        })
    return in_maps


def kernel(x, Wq, Wk, Wv, Wo, _trace=False):
    nc = _get_nc()
    in_maps = _host_inputs(np.asarray(x), np.asarray(Wq), np.asarray(Wk),
                           np.asarray(Wv), np.asarray(Wo))
    res = run_bass_kernel_spmd(nc, in_maps, core_ids=list(range(N_CORES)),
                               trace=_trace)
    out = np.zeros((L, D), np.float32)
    for c in range(N_CORES):
        out += res.results[c]["out"]
    if _trace:
        _CACHE["last_results"] = res
    return out.reshape(1, L, D)
